# revision 25
# baseline (speedup 1.0000x reference)
"""HGNNPConv Trainium2 kernel (8 NeuronCores, SPMD).

Math (equivalent reformulation of the reference):
  Xe_raw[e] = mean_{i: e_idx[i]=e} X[v_idx[i]]              (v2e, softmax of ones = 1/deg)
  Xe_p      = Xe_raw @ W.T + b                              (GEMM on 4000 edges, not 20000 verts)
  Xv[v]     = sum_i wn_i * Xe_p[e_idx[i]],  wn_i = exp(w_i)/sum_{v} exp(w)
              (wn precomputed on host -> no on-chip denominator pass)
  out       = relu(Xv)
Empty edges get a spurious +b in Xe_p but are never referenced downstream
(an edge appearing in phase 2 has >=1 incidence, hence deg>=1 in phase 1).

Sharding: phase 1 by destination edge (500/core), edge-level GEMM per core,
AllGather of the projected edge table (1MB/core), phase 2 by destination
vertex (2500/core). Per-destination-window weighted one-hot selection
matrices (built on DVE from iota) reduce gathered rows on the PE into PSUM.
Phase-1 gather table is fp8-e3m4 (X pre-scaled by 2), phase-2 table bf16,
output bf16 (upcast on host).
"""

import os
from contextlib import ExitStack

import numpy as np
import ml_dtypes

# ---------------------------------------------------------------- config ---
NCORES = 8
NV, NE, NNZ, CH = 20000, 4000, 160000, 512
GATHER_BF16 = os.environ.get("KERNEL_F32", "") == ""  # bf16 tables+matmuls by default
P1_DT = os.environ.get("KERNEL_P1_DT", "f8")   # phase-1 gather table dtype
P2_DT = os.environ.get("KERNEL_P2_DT", "bf16")  # phase-2 gather table dtype
OUT_DT = os.environ.get("KERNEL_OUT_DT", "bf16")
P1_SCALE = 2.0   # X pre-scale for fp8-e3m4 range use (exact power of 2)
GRP = 5          # gather chunks (of 128 idxs) per dma_gather call
TRACE = os.environ.get("BASS_TRACE", "") != ""


def _mydt(mybir, name):
    return {"f8": mybir.dt.float8e3, "bf16": mybir.dt.bfloat16,
            "f32": mybir.dt.float32}[name]


def _npdt(name):
    return {"f8": ml_dtypes.float8_e3m4, "bf16": ml_dtypes.bfloat16,
            "f32": np.float32}[name]

_last_results = None   # BassKernelResults of the most recent run (for test.py)


# ------------------------------------------------------------------- plan ---
class Plan:
    pass


def _binpack(ids, degs, nbins, cap=128):
    """Pack `ids` into `nbins` bins of <=cap items, balancing sum(degs)."""
    import heapq

    order = np.argsort(-degs, kind="stable")
    bins = [[] for _ in range(nbins)]
    loads = [0] * nbins
    heap = [(0, b) for b in range(nbins)]
    heapq.heapify(heap)
    for t in order:
        popped = []
        while True:
            load, b = heapq.heappop(heap)
            if len(bins[b]) < cap:
                break
            popped.append((load, b))
        for p in popped:
            heapq.heappush(heap, p)
        bins[b].append(int(ids[t]))
        loads[b] = load + int(degs[t])
        heapq.heappush(heap, (loads[b], b))
    return bins, loads


def _csr(idx, n):
    order = np.argsort(idx, kind="stable").astype(np.int64)
    deg = np.bincount(idx, minlength=n).astype(np.int64)
    starts = np.zeros(n + 1, np.int64)
    np.cumsum(deg, out=starts[1:])
    return order, deg, starts


def _phase_arrays(bins_per_core, order, starts, W, nw, idx_of_inc, w_of_inc, loc_dtype=np.float32):
    """Per-core flat arrays for one aggregation phase.

    Returns (gidx[int16, 128*W*nw], loc[f32], wsel[f32], members) where slot
    i = (chunk c = i//128, p = i%128), chunk c belongs to window c//W.
    members[w][j] = destination id at window w row j.
    """
    C = nw * W
    L = C * 128
    gidx = np.zeros(L, np.int16)
    loc = np.full(L, -1.0, loc_dtype)
    wsel = np.zeros(L, np.float32)
    members = []
    for w, bin_ids in enumerate(bins_per_core):
        incs = []
        locs = []
        for j, d in enumerate(bin_ids):
            seg = order[starts[d]:starts[d + 1]]
            incs.append(seg)
            locs.append(np.full(len(seg), j, loc_dtype))
        incs = np.concatenate(incs) if incs else np.zeros(0, np.int64)
        locs = np.concatenate(locs) if locs else np.zeros(0, loc_dtype)
        n = len(incs)
        assert n <= W * 128, (n, W)
        o = w * W * 128
        gidx[o:o + n] = idx_of_inc[incs]
        loc[o:o + n] = locs
        wsel[o:o + n] = w_of_inc[incs]
        members.append(bin_ids)
    return gidx, loc, wsel, members


def _wrap_idx(flat):
    """int16 flat[i] -> [128, len/16] with value i at [i%16, i//16], replicated."""
    a = flat.reshape(-1, 16).T  # [16, L/16]
    return np.ascontiguousarray(np.tile(a, (8, 1)))


def _pack(flat, C):
    """flat[c*128+p] -> [128, C]"""
    return np.ascontiguousarray(flat.reshape(C, 128).T)


def make_plan(v_idx, e_idx, e2v_weight, nv=NV, ne=NE, ch=CH, ncores=NCORES):
    P = Plan()
    P.nv, P.ne, P.ch, P.ncores = nv, ne, ch, ncores
    epc, vpc = ne // ncores, nv // ncores
    P.epc, P.vpc = epc, vpc

    order_e, deg_e, starts_e = _csr(e_idx, ne)
    order_v, deg_v, starts_v = _csr(v_idx, nv)
    inv_deg = np.zeros(ne, np.float32)
    nz = deg_e > 0
    inv_deg[nz] = (np.float32(1.0) / deg_e[nz].astype(np.float32))

    nb1 = -(-epc // 128)
    nb2 = -(-vpc // 128)
    bins1, bins2 = [], []
    w1max = w2max = 0
    for k in range(ncores):
        eids = np.arange(k * epc, (k + 1) * epc)
        b, loads = _binpack(eids, deg_e[eids], nb1)
        bins1.append(b)
        w1max = max(w1max, max(loads))
        vids = np.arange(k * vpc, (k + 1) * vpc)
        b, loads = _binpack(vids, deg_v[vids], nb2)
        bins2.append(b)
        w2max = max(w2max, max(loads))
    P.NW1, P.NW2 = nb1, nb2
    P.W1 = -(-w1max // 128)
    P.W2 = -(-w2max // 128)
    P.C1 = P.NW1 * P.W1
    P.C2 = P.NW2 * P.W2

    # phase-1 arrays + edge position map
    pos = np.zeros(ne, np.int64)
    P.p1 = []
    for k in range(ncores):
        gidx, loc, wsel, members = _phase_arrays(
            bins1[k], order_e, starts_e, P.W1, P.NW1, v_idx.astype(np.int64),
            inv_deg[e_idx.astype(np.int64)])
        P.p1.append((gidx, loc, wsel))
        for w, bin_ids in enumerate(members):
            for j, e in enumerate(bin_ids):
                pos[e] = k * P.NW1 * 128 + w * 128 + j
    assert pos.max() < 32768

    # phase-2 arrays + output row map. Softmax weights are fully normalized on
    # the host (exp / per-vertex sum), so the kernel needs no denominator pass.
    expw = np.exp(e2v_weight.astype(np.float64))
    den = np.zeros(nv, np.float64)
    np.add.at(den, v_idx, expw)
    wnorm = (expw / den[v_idx.astype(np.int64)]).astype(np.float32)
    P.p2 = []
    P.vmap = []
    for k in range(ncores):
        gidx, loc, wsel, members = _phase_arrays(
            bins2[k], order_v, starts_v, P.W2, P.NW2, pos[e_idx.astype(np.int64)],
            wnorm)
        P.p2.append((gidx, loc, wsel))
        vm = np.full(P.NW2 * 128, -1, np.int64)
        for w, bin_ids in enumerate(members):
            vm[w * 128:w * 128 + len(bin_ids)] = bin_ids
        P.vmap.append(vm)
    return P


# ---------------------------------------------------------------- builder ---
def build_nc(P, bf16=GATHER_BF16, spmd=True, reps=1, grp=GRP, gbufs=6,
             nqueues=1, p1_dt=P1_DT, p2_dt=P2_DT, out_dt=OUT_DT):
    import concourse.bacc as bacc
    import concourse.mybir as mybir
    import concourse.tile as tile

    f32 = mybir.dt.float32
    dt_g = mybir.dt.bfloat16 if bf16 else f32   # sel matrices + GEMM operands
    dt_p1 = _mydt(mybir, p1_dt)
    dt_p2 = _mydt(mybir, p2_dt)
    dt_out = _mydt(mybir, out_dt)
    eq, mul, mx, add = (mybir.AluOpType.is_equal, mybir.AluOpType.mult,
                        mybir.AluOpType.max, mybir.AluOpType.add)
    ch, KT = P.ch, P.ch // 128

    nc = bacc.Bacc("TRN2", target_bir_lowering=False, debug=False,
                   num_devices=P.ncores if spmd else 1,
                   num_swdge_queues=nqueues)

    XT = nc.dram_tensor("xt", [P.nv, ch], dt_p1, kind="ExternalInput")
    WT = nc.dram_tensor("wt", [128, KT, ch], dt_g, kind="ExternalInput")
    BREP = nc.dram_tensor("brep", [128, ch], f32, kind="ExternalInput")
    IOTA = nc.dram_tensor("iota", [128, 128], f32, kind="ExternalInput")
    IDENT = nc.dram_tensor("ident", [128, 128], f32, kind="ExternalInput")
    G1IDX = nc.dram_tensor("g1idx", [128, P.C1 * 8], mybir.dt.int16, kind="ExternalInput")
    ELOC1 = nc.dram_tensor("eloc1", [128, P.C1], f32, kind="ExternalInput")
    WSEL1 = nc.dram_tensor("wsel1", [128, P.C1], f32, kind="ExternalInput")
    G2IDX = nc.dram_tensor("g2idx", [128, P.C2 * 8], mybir.dt.int16, kind="ExternalInput")
    VLOC2 = nc.dram_tensor("vloc2", [128, P.C2], f32, kind="ExternalInput")
    W2RAW = nc.dram_tensor("w2raw", [128, P.C2], f32, kind="ExternalInput")

    ner1 = P.NW1 * 128
    CCIN = nc.dram_tensor("ccin", [ner1, ch], dt_p2)
    CCOUT = nc.dram_tensor("ccout", [P.ncores * ner1, ch], dt_p2, addr_space="Shared")
    OUT = nc.dram_tensor("out", [P.NW2 * 128, ch], dt_out, kind="ExternalOutput")

    with tile.TileContext(nc) as tc, ExitStack() as ctx:
        const = ctx.enter_context(tc.tile_pool(name="const", bufs=1))
        gpool = ctx.enter_context(tc.tile_pool(name="g", bufs=gbufs))
        selp = ctx.enter_context(tc.tile_pool(name="selp", bufs=6))
        psum = ctx.enter_context(tc.tile_pool(name="ps", bufs=2, space="PSUM"))
        sbp = ctx.enter_context(tc.tile_pool(name="sbp", bufs=2))

        def cload(dram, shape, dt, tag):
            t = const.tile(shape, dt, tag=tag)
            nc.sync.dma_start(t[:], dram[:])
            return t

        wt_t = cload(WT, [128, KT, ch], dt_g, "wt")
        brep_t = cload(BREP, [128, ch], f32, "brep")
        iota_t = cload(IOTA, [128, 128], f32, "iota")
        ident_t = cload(IDENT, [128, 128], f32, "ident")
        g1idx_t = cload(G1IDX, [128, P.C1 * 8], mybir.dt.int16, "g1idx")
        eloc1_t = cload(ELOC1, [128, P.C1], f32, "eloc1")
        wsel1_t = cload(WSEL1, [128, P.C1], f32, "wsel1")
        g2idx_t = cload(G2IDX, [128, P.C2 * 8], mybir.dt.int16, "g2idx")
        vloc2_t = cload(VLOC2, [128, P.C2], f32, "vloc2")
        w2raw_t = cload(W2RAW, [128, P.C2], f32, "w2raw")

        # ---------------- phase 1: v2e mean aggregation --------------------
        def agg_phase(src_ap, gidx_t, loc_t, w_t, C, W, gtag, chunk_cb, win_cb,
                      dt_tab):
            pw = None
            for g0 in range(0, C, grp):
                n = min(grp, C - g0)
                gt = gpool.tile([128, n, ch], dt_tab, tag=gtag)
                nc.gpsimd.dma_gather(
                    gt[:], src_ap, gidx_t[:, g0 * 8:(g0 + n) * 8],
                    n * 128, n * 128, ch, queue_num=(g0 // grp) % nqueues)
                for j in range(n):
                    c = g0 + j
                    w, cw = divmod(c, W)
                    sel = selp.tile([128, 128], dt_g, tag="sel")
                    nc.vector.tensor_scalar(
                        sel[:], iota_t[:], loc_t[:, c:c + 1], w_t[:, c:c + 1],
                        op0=eq, op1=mul)
                    if cw == 0:
                        pw = psum.tile([128, ch], f32, tag="win")
                    chunk_cb(pw, sel, gt, j, w, cw, cw == W - 1)
                    if cw == W - 1:
                        win_cb(pw, w)

        def p1_chunk(pw, sel, gt, j, w, cw, last):
            nc.tensor.matmul(pw[:], sel[:], gt[:, j, :],
                             start=(cw == 0), stop=last)

        def p1_win(pw, w):
            # window w's edge rows are complete: transpose, project, and ship
            # its CCIN slice immediately so GEMM overlaps later p1 gathers.
            xe_w = sbp.tile([128, ch], f32, tag="xew", name="xew")
            nc.vector.tensor_copy(xe_w[:], pw[:])
            xeT_w = sbp.tile([128, KT, 128], dt_g, tag="xeTw", name="xeTw")
            for k in range(KT):
                pt = psum.tile([128, 128], f32, tag="aux", name="pt")
                nc.tensor.transpose(pt[:], xe_w[:, k * 128:(k + 1) * 128],
                                    ident_t[:])
                nc.vector.tensor_copy(xeT_w[:, k, :], pt[:])
            pg = psum.tile([128, ch], f32, tag="aux", name="pg")
            for k in range(KT):
                nc.tensor.matmul(pg[:], xeT_w[:, k, :], wt_t[:, k, :],
                                 start=(k == 0), stop=(k == KT - 1))
            xep = sbp.tile([128, ch], dt_p2, tag="xep", name="xep")
            nc.vector.tensor_tensor(xep[:], pg[:], brep_t[:], op=add)
            nc.sync.dma_start(CCIN[w * 128:(w + 1) * 128, :], xep[:])

        def p2_chunk(pw, sel, gt, j, w, cw, last):
            nc.tensor.matmul(pw[:], sel[:], gt[:, j, :],
                             start=(cw == 0), stop=last)

        def p2_win(pw, w):
            # weights pre-normalized on host: just relu + store
            ow = sbp.tile([128, ch], dt_out, tag="ow", name="ow")
            nc.vector.tensor_scalar(ow[:], pw[:], 1.0, 0.0, op0=mul, op1=mx)
            nc.sync.dma_start(OUT[w * 128:(w + 1) * 128, :], ow[:])

        for _rep in range(reps):
            agg_phase(XT[:], g1idx_t, eloc1_t, wsel1_t, P.C1, P.W1, "g1",
                      p1_chunk, p1_win, dt_p1)

            if spmd:
                nc.gpsimd.collective_compute(
                    "AllGather", mybir.AluOpType.bypass,
                    replica_groups=[list(range(P.ncores))],
                    ins=[CCIN[:]], outs=[CCOUT[:]])
            else:  # single-core cost-model build: stand-in for the AllGather
                nc.sync.dma_start(CCOUT[0:ner1, :], CCIN[:])

            # phase 2: e2v aggregation (sel weights pre-normalized on host)
            agg_phase(CCOUT[:], g2idx_t, vloc2_t, w2raw_t, P.C2, P.W2, "g2",
                      p2_chunk, p2_win, dt_p2)

    nc.compile()
    return nc


# ------------------------------------------------------------------ runner ---
def make_in_maps(P, X, W, b, bf16=GATHER_BF16, p1_dt=P1_DT):
    npdt = ml_dtypes.bfloat16 if bf16 else np.float32
    np_p1 = _npdt(p1_dt)
    s1 = P1_SCALE if p1_dt == "f8" else 1.0
    KT = P.ch // 128
    xt = np.ascontiguousarray((X * s1).astype(np_p1))
    wt = np.ascontiguousarray(
        W.T.reshape(KT, 128, P.ch).transpose(1, 0, 2).astype(npdt))
    brep = np.ascontiguousarray(np.broadcast_to(b.astype(np.float32), (128, P.ch)))
    iota = np.ascontiguousarray(
        np.broadcast_to(np.arange(128, dtype=np.float32), (128, 128)))
    ident = np.eye(128, dtype=np.float32)
    in_maps = []
    for k in range(P.ncores):
        g1, l1, w1 = P.p1[k]
        g2, l2, w2 = P.p2[k]
        in_maps.append({
            "xt": xt, "wt": wt, "brep": brep, "iota": iota, "ident": ident,
            "g1idx": _wrap_idx(g1), "eloc1": _pack(l1, P.C1),
            "wsel1": _pack(w1, P.C1) / np.float32(s1),
            "g2idx": _wrap_idx(g2), "vloc2": _pack(l2, P.C2), "w2raw": _pack(w2, P.C2),
        })
    return in_maps


def assemble(P, shards):
    out = np.zeros((P.nv, P.ch), np.float32)
    for k in range(P.ncores):
        vm = P.vmap[k]
        m = vm >= 0
        out[vm[m]] = shards[k][m].astype(np.float32)
    return out


_nc_cache = {}


def kernel(X, W, b, e2v_weight, v_idx, e_idx):
    global _last_results
    from concourse.bass_utils import run_bass_kernel_spmd

    P = make_plan(v_idx, e_idx, e2v_weight)
    key = (P.C1, P.C2, P.W1, P.W2, GATHER_BF16, P1_DT, P2_DT, OUT_DT)
    if key not in _nc_cache:
        _nc_cache[key] = build_nc(P)
    nc = _nc_cache[key]
    in_maps = make_in_maps(P, X, W, b)
    res = run_bass_kernel_spmd(nc, in_maps, list(range(P.ncores)), trace=TRACE)
    _last_results = res
    shards = [res.results[k]["out"] for k in range(P.ncores)]
    return assemble(P, shards)



# revision 46
# speedup vs baseline: 1.0259x; 1.0259x over previous
"""HGNNPConv Trainium2 kernel (8 NeuronCores, SPMD).

Math (equivalent reformulation of the reference):
  Xe_raw[e] = mean_{i: e_idx[i]=e} X[v_idx[i]]              (v2e, softmax of ones = 1/deg)
  Xe_p      = Xe_raw @ W.T + b                              (GEMM on 4000 edges, not 20000 verts)
  Xv[v]     = sum_i wn_i * Xe_p[e_idx[i]],  wn_i = exp(w_i)/sum_{v} exp(w)
              (wn precomputed on host -> no on-chip denominator pass)
  out       = relu(Xv)
Empty edges get a spurious +b in Xe_p but are never referenced downstream
(an edge appearing in phase 2 has >=1 incidence, hence deg>=1 in phase 1).

Sharding: phase 1 by destination edge (500/core), edge-level GEMM per core,
AllGather of the projected edge table (1MB/core), phase 2 by destination
vertex (2500/core). Per-destination-window weighted one-hot selection
matrices (built on DVE from iota) reduce gathered rows on the PE into PSUM.
Phase-1 gather table is fp8-e3m4 (X pre-scaled by 2), phase-2 table bf16,
output bf16 (upcast on host).
"""

import os
from contextlib import ExitStack

import numpy as np
import ml_dtypes

# ---------------------------------------------------------------- config ---
NCORES = 8
NV, NE, NNZ, CH = 20000, 4000, 160000, 512
GATHER_BF16 = os.environ.get("KERNEL_F32", "") == ""  # bf16 tables+matmuls by default
P1_DT = os.environ.get("KERNEL_P1_DT", "f8")   # phase-1 gather table dtype
P2_DT = os.environ.get("KERNEL_P2_DT", "bf16")  # phase-2 gather table dtype
OUT_DT = os.environ.get("KERNEL_OUT_DT", "bf16")
FUSE = os.environ.get("KERNEL_FUSE", "pre")    # "post": GEMM after p2 agg
P1_SCALE = 2.0   # X pre-scale for fp8-e3m4 range use (exact power of 2)
P2_SCALE = 8.0   # Xe_raw pre-scale for the fp8 edge table (post mode)
GRP = 5          # gather chunks (of 128 idxs) per dma_gather call
TRACE = os.environ.get("BASS_TRACE", "") != ""


def _mydt(mybir, name):
    return {"f8": mybir.dt.float8e3, "bf16": mybir.dt.bfloat16,
            "f32": mybir.dt.float32}[name]


def _npdt(name):
    return {"f8": ml_dtypes.float8_e3m4, "bf16": ml_dtypes.bfloat16,
            "f32": np.float32}[name]

_last_results = None   # BassKernelResults of the most recent run (for test.py)


# ------------------------------------------------------------------- plan ---
class Plan:
    pass


def _binpack(ids, degs, nbins, cap=128):
    """Pack `ids` into `nbins` bins of <=cap items, balancing sum(degs)."""
    import heapq

    order = np.argsort(-degs, kind="stable")
    bins = [[] for _ in range(nbins)]
    loads = [0] * nbins
    heap = [(0, b) for b in range(nbins)]
    heapq.heapify(heap)
    for t in order:
        popped = []
        while True:
            load, b = heapq.heappop(heap)
            if len(bins[b]) < cap:
                break
            popped.append((load, b))
        for p in popped:
            heapq.heappush(heap, p)
        bins[b].append(int(ids[t]))
        loads[b] = load + int(degs[t])
        heapq.heappush(heap, (loads[b], b))
    return bins, loads


def _csr(idx, n):
    order = np.argsort(idx, kind="stable").astype(np.int64)
    deg = np.bincount(idx, minlength=n).astype(np.int64)
    starts = np.zeros(n + 1, np.int64)
    np.cumsum(deg, out=starts[1:])
    return order, deg, starts


def _pair_window(src, locs, ws):
    """Greedy within-window dedup: incidences sharing a source row become one
    gathered slot with two (loc, w) hots.  Returns (src', loc_a, w_a, loc_b,
    w_b) with pair slots FIRST; singles have loc_b = -1 / w_b = 0."""
    order = np.argsort(src, kind="stable")
    src, locs, ws = src[order], locs[order], ws[order]
    pa, pb, sg = [], [], []
    i, n = 0, len(src)
    while i < n:
        j = i
        while j < n and src[j] == src[i]:
            j += 1
        k = i
        while k + 1 < j:
            pa.append(k); pb.append(k + 1); k += 2
        if k < j:
            sg.append(k)
        i = j
    pa, pb, sg = np.array(pa, np.int64), np.array(pb, np.int64), np.array(sg, np.int64)
    src2 = np.concatenate([src[pa], src[sg]]) if len(pa) else src[sg]
    la = np.concatenate([locs[pa], locs[sg]]) if len(pa) else locs[sg]
    wa = np.concatenate([ws[pa], ws[sg]]) if len(pa) else ws[sg]
    lb = np.concatenate([locs[pb], np.full(len(sg), -1.0, locs.dtype)]) if len(pa) \
        else np.full(len(sg), -1.0, locs.dtype)
    wb = np.concatenate([ws[pb], np.zeros(len(sg), ws.dtype)]) if len(pa) \
        else np.zeros(len(sg), ws.dtype)
    return src2, la, wa, lb, wb, len(pa)


def _phase_windows(bins_per_core, order, starts, idx_of_inc, w_of_inc,
                   loc_dtype=np.float32, pair=True):
    """Per-window slot lists for one core of one phase, after source dedup.

    Returns (wins, wmax): wins[w] = (src, loc_a, w_a, loc_b, w_b); pair slots
    (loc_b >= 0) come first within each window.
    """
    wins = []
    wmax = 0
    for bin_ids in bins_per_core:
        incs = []
        locs = []
        for j, d in enumerate(bin_ids):
            seg = order[starts[d]:starts[d + 1]]
            incs.append(seg)
            locs.append(np.full(len(seg), j, loc_dtype))
        incs = np.concatenate(incs) if incs else np.zeros(0, np.int64)
        locs = np.concatenate(locs) if locs else np.zeros(0, loc_dtype)
        src = idx_of_inc[incs]
        ws = w_of_inc[incs].astype(np.float32)
        if pair and len(src):
            src, la, wa, lb, wb, _ = _pair_window(src, locs, ws)
        else:
            la, wa = locs, ws
            lb = np.full(len(src), -1.0, loc_dtype)
            wb = np.zeros(len(src), np.float32)
        wins.append((src, la, wa, lb, wb))
        wmax = max(wmax, len(src))
    return wins, wmax


def _layout(wins, W, nw):
    """Flat slot arrays: slot i = (chunk i//128, partition i%128); chunk c
    belongs to window c//W."""
    L = nw * W * 128
    gidx = np.zeros(L, np.int16)
    loc = np.full(L, -1.0, np.float32)
    wsel = np.zeros(L, np.float32)
    locb = np.full(L, -1.0, np.float32)
    wselb = np.zeros(L, np.float32)
    for w, (src, la, wa, lb, wb) in enumerate(wins):
        n = len(src)
        o = w * W * 128
        gidx[o:o + n] = src
        loc[o:o + n] = la
        wsel[o:o + n] = wa
        locb[o:o + n] = lb
        wselb[o:o + n] = wb
    return gidx, loc, wsel, locb, wselb


def _wrap_idx(flat):
    """int16 flat[i] -> [128, len/16] with value i at [i%16, i//16], replicated."""
    a = flat.reshape(-1, 16).T  # [16, L/16]
    return np.ascontiguousarray(np.tile(a, (8, 1)))


def _pack(flat, C):
    """flat[c*128+p] -> [128, C]"""
    return np.ascontiguousarray(flat.reshape(C, 128).T)


def _dedup_slots(srcs):
    """#gather slots for a window's source list after pairing."""
    if not len(srcs):
        return 0
    _, cnt = np.unique(srcs, return_counts=True)
    return int(((cnt + 1) // 2).sum())


def _repair_bins(bins, order, starts, idx_of_inc, cap=128, iters=400):
    """Greedy rebalance: move members out of the window with the most
    post-dedup slots into the one with the fewest (respecting the member
    cap), to minimize max slots per window."""
    srcs = [
        [idx_of_inc[order[starts[d]:starts[d + 1]]] for d in b] for b in bins
    ]

    def slots(w):
        return _dedup_slots(np.concatenate(srcs[w]) if srcs[w] else
                            np.zeros(0, np.int64))

    cur = [slots(w) for w in range(len(bins))]
    for _ in range(iters):
        hot = int(np.argmax(cur))
        order_cold = np.argsort(cur)
        moved = False
        for cold in order_cold:
            if cold == hot or len(bins[cold]) >= cap:
                continue
            # move the member with the smallest segment out of `hot`
            j = int(np.argmin([len(s) for s in srcs[hot]]))
            bins[cold].append(bins[hot].pop(j))
            srcs[cold].append(srcs[hot].pop(j))
            new_hot, new_cold = slots(hot), slots(cold)
            if max(new_hot, new_cold) >= cur[hot]:
                # revert: no improvement
                bins[hot].append(bins[cold].pop())
                srcs[hot].append(srcs[cold].pop())
                continue
            cur[hot], cur[cold] = new_hot, new_cold
            moved = True
            break
        if not moved:
            break
    return bins


def make_plan(v_idx, e_idx, e2v_weight, nv=NV, ne=NE, ch=CH, ncores=NCORES):
    P = Plan()
    P.nv, P.ne, P.ch, P.ncores = nv, ne, ch, ncores
    epc, vpc = ne // ncores, nv // ncores
    P.epc, P.vpc = epc, vpc

    order_e, deg_e, starts_e = _csr(e_idx, ne)
    order_v, deg_v, starts_v = _csr(v_idx, nv)
    inv_deg = np.zeros(ne, np.float32)
    nz = deg_e > 0
    inv_deg[nz] = (np.float32(1.0) / deg_e[nz].astype(np.float32))

    nb1 = -(-epc // 128)
    nb2 = -(-vpc // 128)
    v_of_inc = v_idx.astype(np.int64)
    e_of_inc = e_idx.astype(np.int64)
    # balance destinations across cores globally (assignment is free — pos /
    # vmap carry it), then binpack windows within each core and rebalance for
    # post-dedup slot counts.
    cores_e, _ = _binpack(np.arange(ne), deg_e, ncores, cap=nb1 * 128)
    bins1 = []
    for k in range(ncores):
        eids = np.asarray(cores_e[k])
        b, _ = _binpack(eids, deg_e[eids], nb1)
        bins1.append(_repair_bins(b, order_e, starts_e, v_of_inc))

    # p2 window count: an extra window can admit a smaller W2 (less gather
    # padding) once dedup shrinks the per-window loads — pick the best.
    best = None
    for nb2c in (nb2, nb2 + 1):
        cores_v, _ = _binpack(np.arange(nv), deg_v, ncores, cap=nb2c * 128)
        cand = []
        wmax = 0
        for k in range(ncores):
            vids = np.asarray(cores_v[k])
            b, _ = _binpack(vids, deg_v[vids], nb2c)
            b = _repair_bins(b, order_v, starts_v, e_of_inc)
            cand.append(b)
            for bb in b:
                wmax = max(wmax, _dedup_slots(np.concatenate(
                    [e_of_inc[order_v[starts_v[d]:starts_v[d + 1]]]
                     for d in bb]) if bb else np.zeros(0, np.int64)))
        W2c = -(-wmax // 128)
        if best is None or nb2c * W2c < best[0] * best[1]:
            best = (nb2c, W2c, cand)
    nb2, _, bins2 = best
    P.NW1, P.NW2 = nb1, nb2

    # phase-1 windows (dedup within window) + edge position map
    pos = np.zeros(ne, np.int64)
    wins1 = []
    w1max = 0
    for k in range(ncores):
        wins, wmax = _phase_windows(
            bins1[k], order_e, starts_e, v_idx.astype(np.int64),
            inv_deg[e_idx.astype(np.int64)])
        wins1.append(wins)
        w1max = max(w1max, wmax)
        for w, bin_ids in enumerate(bins1[k]):
            for j, e in enumerate(bin_ids):
                pos[e] = k * P.NW1 * 128 + w * 128 + j
    assert pos.max() < 32768
    P.W1 = -(-w1max // 128)
    P.C1 = P.NW1 * P.W1
    P.p1 = [_layout(wins, P.W1, P.NW1) for wins in wins1]

    # phase-2 windows + output row map. Softmax weights are fully normalized
    # on the host (exp / per-vertex sum), so the kernel needs no denominator
    # pass.
    expw = np.exp(e2v_weight.astype(np.float64))
    den = np.zeros(nv, np.float64)
    np.add.at(den, v_idx, expw)
    wnorm = (expw / den[v_idx.astype(np.int64)]).astype(np.float32)
    wins2 = []
    w2max = 0
    P.vmap = []
    for k in range(ncores):
        wins, wmax = _phase_windows(
            bins2[k], order_v, starts_v, pos[e_idx.astype(np.int64)], wnorm)
        wins2.append(wins)
        w2max = max(w2max, wmax)
        vm = np.full(P.NW2 * 128, -1, np.int64)
        for w, bin_ids in enumerate(bins2[k]):
            vm[w * 128:w * 128 + len(bin_ids)] = bin_ids
        # deg-0 vertices never receive contributions; drop them from the
        # output map so any on-chip garbage (e.g. a stray +b) is discarded.
        vme = vm[vm >= 0]
        vm[vm >= 0] = np.where(deg_v[vme] > 0, vme, -1)
        P.vmap.append(vm)
    P.W2 = -(-w2max // 128)
    P.C2 = P.NW2 * P.W2
    P.p2 = [_layout(wins, P.W2, P.NW2) for wins in wins2]
    return P


# ---------------------------------------------------------------- builder ---
def build_nc(P, bf16=GATHER_BF16, spmd=True, reps=1, grp=GRP, gbufs=6,
             nqueues=1, p1_dt=P1_DT, p2_dt=P2_DT, out_dt=OUT_DT, fuse=FUSE):
    import concourse.bacc as bacc
    import concourse.mybir as mybir
    import concourse.tile as tile

    f32 = mybir.dt.float32
    dt_g = mybir.dt.bfloat16 if bf16 else f32   # sel matrices + GEMM operands
    dt_p1 = _mydt(mybir, p1_dt)
    dt_p2 = _mydt(mybir, p2_dt)
    dt_out = _mydt(mybir, out_dt)
    eq, mul, mx, add = (mybir.AluOpType.is_equal, mybir.AluOpType.mult,
                        mybir.AluOpType.max, mybir.AluOpType.add)
    ch, KT = P.ch, P.ch // 128
    post = fuse == "post"

    nc = bacc.Bacc("TRN2", target_bir_lowering=False, debug=False,
                   num_devices=P.ncores if spmd else 1,
                   num_swdge_queues=nqueues)

    XT = nc.dram_tensor("xt", [P.nv, ch], dt_p1, kind="ExternalInput")
    WT = nc.dram_tensor("wt", [128, KT, ch], dt_g, kind="ExternalInput")
    BT = nc.dram_tensor("bt", [1, ch], dt_g, kind="ExternalInput")
    IOTA = nc.dram_tensor("iota", [128, 128], dt_g, kind="ExternalInput")
    IDENT = nc.dram_tensor("ident", [128, 128], dt_g, kind="ExternalInput")
    G1IDX = nc.dram_tensor("g1idx", [128, P.C1 * 8], mybir.dt.int16, kind="ExternalInput")
    ELOC1 = nc.dram_tensor("eloc1", [128, P.C1], dt_g, kind="ExternalInput")
    WSEL1 = nc.dram_tensor("wsel1", [128, P.C1], dt_g, kind="ExternalInput")
    ELOC1B = nc.dram_tensor("eloc1b", [128, P.C1], dt_g, kind="ExternalInput")
    WSEL1B = nc.dram_tensor("wsel1b", [128, P.C1], dt_g, kind="ExternalInput")
    G2IDX = nc.dram_tensor("g2idx", [128, P.C2 * 8], mybir.dt.int16, kind="ExternalInput")
    VLOC2 = nc.dram_tensor("vloc2", [128, P.C2], dt_g, kind="ExternalInput")
    W2RAW = nc.dram_tensor("w2raw", [128, P.C2], dt_g, kind="ExternalInput")
    VLOC2B = nc.dram_tensor("vloc2b", [128, P.C2], dt_g, kind="ExternalInput")
    W2RAWB = nc.dram_tensor("w2rawb", [128, P.C2], dt_g, kind="ExternalInput")

    ner1 = P.NW1 * 128
    CCIN = nc.dram_tensor("ccin", [ner1, ch], dt_p2)
    CCOUT = nc.dram_tensor("ccout", [P.ncores * ner1, ch], dt_p2, addr_space="Shared")
    OUT = nc.dram_tensor("out", [P.NW2 * 128, ch], dt_out, kind="ExternalOutput")

    with tile.TileContext(nc) as tc, ExitStack() as ctx:
        const = ctx.enter_context(tc.tile_pool(name="const", bufs=1))
        gpool = ctx.enter_context(tc.tile_pool(name="g", bufs=gbufs))
        selp = ctx.enter_context(tc.tile_pool(name="selp", bufs=6))
        psum = ctx.enter_context(tc.tile_pool(name="ps", bufs=2, space="PSUM"))
        sbp = ctx.enter_context(tc.tile_pool(name="sbp", bufs=2))

        def cload(dram, shape, dt, tag):
            t = const.tile(shape, dt, tag=tag)
            nc.sync.dma_start(t[:], dram[:])
            return t

        wt_t = cload(WT, [128, KT, ch], dt_g, "wt")
        bt_t = cload(BT, [1, ch], dt_g, "bt")
        iota_t = cload(IOTA, [128, 128], f32, "iota")
        ident_t = cload(IDENT, [128, 128], f32, "ident")
        g1idx_t = cload(G1IDX, [128, P.C1 * 8], mybir.dt.int16, "g1idx")
        eloc1_t = cload(ELOC1, [128, P.C1], f32, "eloc1")
        wsel1_t = cload(WSEL1, [128, P.C1], f32, "wsel1")
        eloc1b_t = cload(ELOC1B, [128, P.C1], f32, "eloc1b")
        wsel1b_t = cload(WSEL1B, [128, P.C1], f32, "wsel1b")
        g2idx_t = cload(G2IDX, [128, P.C2 * 8], mybir.dt.int16, "g2idx")
        vloc2_t = cload(VLOC2, [128, P.C2], f32, "vloc2")
        w2raw_t = cload(W2RAW, [128, P.C2], f32, "w2raw")
        vloc2b_t = cload(VLOC2B, [128, P.C2], f32, "vloc2b")
        w2rawb_t = cload(W2RAWB, [128, P.C2], f32, "w2rawb")
        ones1_t = const.tile([1, 128], dt_g, tag="ones1")
        nc.vector.memset(ones1_t[:], 1.0)

        # ---------------- gather + one/two-hot reduce ----------------------
        def agg_phase(src_ap, gidx_t, loc_t, w_t, locb_t, wb_t, is2, C, W,
                      gtag, chunk_cb, win_cb, dt_tab):
            pw = None
            for g0 in range(0, C, grp):
                n = min(grp, C - g0)
                gt = gpool.tile([128, n, ch], dt_tab, tag=gtag)
                nc.gpsimd.dma_gather(
                    gt[:], src_ap, gidx_t[:, g0 * 8:(g0 + n) * 8],
                    n * 128, n * 128, ch, queue_num=(g0 // grp) % nqueues)
                for j in range(n):
                    c = g0 + j
                    w, cw = divmod(c, W)
                    sel = selp.tile([128, 128], dt_g, tag="sel")
                    nc.vector.tensor_scalar(
                        sel[:], iota_t[:], loc_t[:, c:c + 1], w_t[:, c:c + 1],
                        op0=eq, op1=mul)
                    if is2[c]:  # dedup chunk: add the second hot
                        selb = selp.tile([128, 128], dt_g, tag="selb")
                        nc.vector.tensor_scalar(
                            selb[:], iota_t[:], locb_t[:, c:c + 1],
                            wb_t[:, c:c + 1], op0=eq, op1=mul)
                        sel2 = selp.tile([128, 128], dt_g, tag="sel2")
                        nc.vector.tensor_tensor(sel2[:], sel[:], selb[:], op=add)
                        sel = sel2
                    if cw == 0:
                        pw = psum.tile([128, ch], f32, tag="win")
                    chunk_cb(pw, sel, gt, j, w, cw, cw == W - 1)
                    if cw == W - 1:
                        win_cb(pw, w)

        def p1_chunk(pw, sel, gt, j, w, cw, last):
            nc.tensor.matmul(pw[:], sel[:], gt[:, j, :],
                             start=(cw == 0), stop=last)

        def gemm_bias(src_t, dst_psum):
            """dst[v/e, co] = src^T blocks @ W.T + 1^T b (K=1 bias matmul)."""
            for k in range(KT):
                nc.tensor.matmul(dst_psum[:], src_t[:, k, :], wt_t[:, k, :],
                                 start=(k == 0), stop=False)
            nc.tensor.matmul(dst_psum[:], ones1_t[:], bt_t[:],
                             start=False, stop=True)

        def transpose_blocks(pw, tag):
            """psum [128, ch] f32 -> sbuf [128, KT, 128] dt_g transposed."""
            t_w = sbp.tile([128, ch], f32, tag=tag + "f", name=tag + "f")
            nc.vector.tensor_copy(t_w[:], pw[:])
            tT_w = sbp.tile([128, KT, 128], dt_g, tag=tag + "T", name=tag + "T")
            for k in range(KT):
                pt = psum.tile([128, 128], f32, tag="aux", name="pt")
                nc.tensor.transpose(pt[:], t_w[:, k * 128:(k + 1) * 128],
                                    ident_t[:])
                nc.vector.tensor_copy(tT_w[:, k, :], pt[:])
            return tT_w

        def p1_win(pw, w):
            # window w's edge rows are complete: ship its CCIN slice
            # immediately so downstream work overlaps later p1 gathers.
            xep = sbp.tile([128, ch], dt_p2, tag="xep", name="xep")
            if post:
                # raw table, scaled for fp8 range; GEMM happens after p2 agg
                nc.vector.tensor_scalar(xep[:], pw[:], float(P2_SCALE), None,
                                        op0=mul)
            else:
                xeT_w = transpose_blocks(pw, "xe")
                pg = psum.tile([128, ch], f32, tag="gemm", name="pg")
                gemm_bias(xeT_w, pg)
                nc.vector.tensor_copy(xep[:], pg[:])
            nc.sync.dma_start(CCIN[w * 128:(w + 1) * 128, :], xep[:])

        def p2_chunk(pw, sel, gt, j, w, cw, last):
            nc.tensor.matmul(pw[:], sel[:], gt[:, j, :],
                             start=(cw == 0), stop=last)

        def p2_win(pw, w):
            if post:
                awT = transpose_blocks(pw, "aw")
                po = psum.tile([128, ch], f32, tag="gemm", name="po")
                gemm_bias(awT, po)
                pw = po
            # weights pre-normalized on host: just relu + store
            ow = sbp.tile([128, ch], dt_out, tag="ow", name="ow")
            nc.vector.tensor_scalar(ow[:], pw[:], 1.0, 0.0, op0=mul, op1=mx)
            nc.sync.dma_start(OUT[w * 128:(w + 1) * 128, :], ow[:])

        # chunks that contain any dedup pair need the second sel pass; the
        # union over cores keeps the SPMD program identical on every core.
        is2_1 = np.zeros(P.C1, bool)
        is2_2 = np.zeros(P.C2, bool)
        for k in range(P.ncores):
            is2_1 |= (_pack(P.p1[k][3], P.C1) >= 0).any(axis=0)
            is2_2 |= (_pack(P.p2[k][3], P.C2) >= 0).any(axis=0)

        for _rep in range(reps):
            agg_phase(XT[:], g1idx_t, eloc1_t, wsel1_t, eloc1b_t, wsel1b_t,
                      is2_1, P.C1, P.W1, "g1", p1_chunk, p1_win, dt_p1)

            if spmd:
                nc.gpsimd.collective_compute(
                    "AllGather", mybir.AluOpType.bypass,
                    replica_groups=[list(range(P.ncores))],
                    ins=[CCIN[:]], outs=[CCOUT[:]])
            else:  # single-core cost-model build: stand-in for the AllGather
                nc.sync.dma_start(CCOUT[0:ner1, :], CCIN[:])

            # phase 2: e2v aggregation (sel weights pre-normalized on host)
            agg_phase(CCOUT[:], g2idx_t, vloc2_t, w2raw_t, vloc2b_t, w2rawb_t,
                      is2_2, P.C2, P.W2, "g2", p2_chunk, p2_win, dt_p2)

    nc.compile()
    return nc


# ------------------------------------------------------------------ runner ---
def make_in_maps(P, X, W, b, bf16=GATHER_BF16, p1_dt=P1_DT, fuse=FUSE):
    npdt = ml_dtypes.bfloat16 if bf16 else np.float32
    np_p1 = _npdt(p1_dt)
    s1 = P1_SCALE if p1_dt == "f8" else 1.0
    s2 = P2_SCALE if fuse == "post" else 1.0
    KT = P.ch // 128
    xt = np.ascontiguousarray((X * s1).astype(np_p1))
    wt = np.ascontiguousarray(
        W.T.reshape(KT, 128, P.ch).transpose(1, 0, 2).astype(npdt))
    bt = np.ascontiguousarray(b.astype(npdt).reshape(1, P.ch))
    iota = np.ascontiguousarray(
        np.broadcast_to(np.arange(128, dtype=np.float32), (128, 128)))
    ident = np.eye(128, dtype=np.float32)
    in_maps = []
    for k in range(P.ncores):
        g1, l1, w1, l1b, w1b = P.p1[k]
        g2, l2, w2, l2b, w2b = P.p2[k]
        in_maps.append({
            "xt": xt, "wt": wt, "bt": bt, "iota": iota, "ident": ident,
            "g1idx": _wrap_idx(g1), "eloc1": _pack(l1, P.C1),
            "wsel1": _pack(w1, P.C1) / np.float32(s1),
            "eloc1b": _pack(l1b, P.C1),
            "wsel1b": _pack(w1b, P.C1) / np.float32(s1),
            "g2idx": _wrap_idx(g2), "vloc2": _pack(l2, P.C2),
            "w2raw": _pack(w2, P.C2) / np.float32(s2),
            "vloc2b": _pack(l2b, P.C2),
            "w2rawb": _pack(w2b, P.C2) / np.float32(s2),
        })
    return in_maps


def assemble(P, shards):
    out = np.zeros((P.nv, P.ch), np.float32)
    for k in range(P.ncores):
        vm = P.vmap[k]
        m = vm >= 0
        out[vm[m]] = shards[k][m].astype(np.float32)
    return out


_nc_cache = {}


def kernel(X, W, b, e2v_weight, v_idx, e_idx):
    global _last_results
    from concourse.bass_utils import run_bass_kernel_spmd

    P = make_plan(v_idx, e_idx, e2v_weight)
    key = (P.C1, P.C2, P.W1, P.W2, GATHER_BF16, P1_DT, P2_DT, OUT_DT, FUSE)
    if key not in _nc_cache:
        _nc_cache[key] = build_nc(P)
    nc = _nc_cache[key]
    in_maps = make_in_maps(P, X, W, b)
    res = run_bass_kernel_spmd(nc, in_maps, list(range(P.ncores)), trace=TRACE)
    _last_results = res
    shards = [res.results[k]["out"] for k in range(P.ncores)]
    return assemble(P, shards)



# revision 57
# speedup vs baseline: 1.0411x; 1.0148x over previous
"""HGNNPConv Trainium2 kernel (8 NeuronCores, SPMD).

Math (equivalent reformulation of the reference):
  Xe_raw[e] = mean_{i: e_idx[i]=e} X[v_idx[i]]              (v2e, softmax of ones = 1/deg)
  Xe_p      = Xe_raw @ W.T + b                              (GEMM on 4000 edges, not 20000 verts)
  Xv[v]     = sum_i wn_i * Xe_p[e_idx[i]],  wn_i = exp(w_i)/sum_{v} exp(w)
              (wn precomputed on host -> no on-chip denominator pass)
  out       = relu(Xv)
Empty edges get a spurious +b in Xe_p but are never referenced downstream
(an edge appearing in phase 2 has >=1 incidence, hence deg>=1 in phase 1).

Sharding: phase 1 by destination edge (500/core), edge-level GEMM per core,
AllGather of the projected edge table (1MB/core), phase 2 by destination
vertex (2500/core). Per-destination-window weighted one-hot selection
matrices (built on DVE from iota) reduce gathered rows on the PE into PSUM.
Phase-1 gather table is fp8-e3m4 (X pre-scaled by 2), phase-2 table bf16,
output bf16 (upcast on host).
"""

import os
from contextlib import ExitStack

import numpy as np
import ml_dtypes

# ---------------------------------------------------------------- config ---
NCORES = 8
NV, NE, NNZ, CH = 20000, 4000, 160000, 512
GATHER_BF16 = os.environ.get("KERNEL_F32", "") == ""  # bf16 tables+matmuls by default
P1_DT = os.environ.get("KERNEL_P1_DT", "f8")   # phase-1 gather table dtype
P2_DT = os.environ.get("KERNEL_P2_DT", "bf16")  # phase-2 gather table dtype
OUT_DT = os.environ.get("KERNEL_OUT_DT", "bf16")
FUSE = os.environ.get("KERNEL_FUSE", "pre")    # "post": GEMM after p2 agg
P1_SCALE = 2.0   # X pre-scale for fp8-e3m4 range use (exact power of 2)
P2_SCALE = 8.0   # Xe_raw pre-scale for the fp8 edge table (post mode)
GRP = 5          # gather chunks (of 128 idxs) per dma_gather call
AG_SLICED = os.environ.get("KERNEL_AG_SLICED", "") != ""  # per-window AllGather
TRACE = os.environ.get("BASS_TRACE", "") != ""


def _mydt(mybir, name):
    return {"f8": mybir.dt.float8e3, "bf16": mybir.dt.bfloat16,
            "f32": mybir.dt.float32}[name]


def _npdt(name):
    return {"f8": ml_dtypes.float8_e3m4, "bf16": ml_dtypes.bfloat16,
            "f32": np.float32}[name]

_last_results = None   # BassKernelResults of the most recent run (for test.py)


# ------------------------------------------------------------------- plan ---
class Plan:
    pass


def _binpack(ids, degs, nbins, cap=128):
    """Pack `ids` into `nbins` bins of <=cap items, balancing sum(degs)."""
    import heapq

    order = np.argsort(-degs, kind="stable")
    bins = [[] for _ in range(nbins)]
    loads = [0] * nbins
    heap = [(0, b) for b in range(nbins)]
    heapq.heapify(heap)
    for t in order:
        popped = []
        while True:
            load, b = heapq.heappop(heap)
            if len(bins[b]) < cap:
                break
            popped.append((load, b))
        for p in popped:
            heapq.heappush(heap, p)
        bins[b].append(int(ids[t]))
        loads[b] = load + int(degs[t])
        heapq.heappush(heap, (loads[b], b))
    return bins, loads


def _csr(idx, n):
    order = np.argsort(idx, kind="stable").astype(np.int64)
    deg = np.bincount(idx, minlength=n).astype(np.int64)
    starts = np.zeros(n + 1, np.int64)
    np.cumsum(deg, out=starts[1:])
    return order, deg, starts


def _pair_window(src, locs, ws):
    """Greedy within-window dedup: incidences sharing a source row become one
    gathered slot with two (loc, w) hots.  Returns (src', loc_a, w_a, loc_b,
    w_b) with pair slots FIRST; singles have loc_b = -1 / w_b = 0."""
    order = np.argsort(src, kind="stable")
    src, locs, ws = src[order], locs[order], ws[order]
    pa, pb, sg = [], [], []
    i, n = 0, len(src)
    while i < n:
        j = i
        while j < n and src[j] == src[i]:
            j += 1
        k = i
        while k + 1 < j:
            pa.append(k); pb.append(k + 1); k += 2
        if k < j:
            sg.append(k)
        i = j
    pa, pb, sg = np.array(pa, np.int64), np.array(pb, np.int64), np.array(sg, np.int64)
    src2 = np.concatenate([src[pa], src[sg]]) if len(pa) else src[sg]
    la = np.concatenate([locs[pa], locs[sg]]) if len(pa) else locs[sg]
    wa = np.concatenate([ws[pa], ws[sg]]) if len(pa) else ws[sg]
    lb = np.concatenate([locs[pb], np.full(len(sg), -1.0, locs.dtype)]) if len(pa) \
        else np.full(len(sg), -1.0, locs.dtype)
    wb = np.concatenate([ws[pb], np.zeros(len(sg), ws.dtype)]) if len(pa) \
        else np.zeros(len(sg), ws.dtype)
    return src2, la, wa, lb, wb, len(pa)


def _phase_windows(bins_per_core, order, starts, idx_of_inc, w_of_inc,
                   loc_dtype=np.float32, pair=True):
    """Per-window slot lists for one core of one phase, after source dedup.

    Returns (wins, wmax): wins[w] = (src, loc_a, w_a, loc_b, w_b); pair slots
    (loc_b >= 0) come first within each window.
    """
    wins = []
    wmax = 0
    for bin_ids in bins_per_core:
        incs = []
        locs = []
        for j, d in enumerate(bin_ids):
            seg = order[starts[d]:starts[d + 1]]
            incs.append(seg)
            locs.append(np.full(len(seg), j, loc_dtype))
        incs = np.concatenate(incs) if incs else np.zeros(0, np.int64)
        locs = np.concatenate(locs) if locs else np.zeros(0, loc_dtype)
        src = idx_of_inc[incs]
        ws = w_of_inc[incs].astype(np.float32)
        if pair and len(src):
            src, la, wa, lb, wb, _ = _pair_window(src, locs, ws)
        else:
            la, wa = locs, ws
            lb = np.full(len(src), -1.0, loc_dtype)
            wb = np.zeros(len(src), np.float32)
        wins.append((src, la, wa, lb, wb))
        wmax = max(wmax, len(src))
    return wins, wmax


def _layout(wins, W, nw):
    """Flat slot arrays: slot i = (chunk i//128, partition i%128); chunk c
    belongs to window c//W."""
    L = nw * W * 128
    gidx = np.zeros(L, np.int16)
    loc = np.full(L, -1.0, np.float32)
    wsel = np.zeros(L, np.float32)
    locb = np.full(L, -1.0, np.float32)
    wselb = np.zeros(L, np.float32)
    for w, (src, la, wa, lb, wb) in enumerate(wins):
        n = len(src)
        o = w * W * 128
        gidx[o:o + n] = src
        loc[o:o + n] = la
        wsel[o:o + n] = wa
        locb[o:o + n] = lb
        wselb[o:o + n] = wb
    return gidx, loc, wsel, locb, wselb


def _wrap_idx(flat):
    """int16 flat[i] -> [128, len/16] with value i at [i%16, i//16], replicated."""
    a = flat.reshape(-1, 16).T  # [16, L/16]
    return np.ascontiguousarray(np.tile(a, (8, 1)))


def _pack(flat, C):
    """flat[c*128+p] -> [128, C]"""
    return np.ascontiguousarray(flat.reshape(C, 128).T)


def _dedup_slots(srcs):
    """#gather slots for a window's source list after pairing."""
    if not len(srcs):
        return 0
    _, cnt = np.unique(srcs, return_counts=True)
    return int(((cnt + 1) // 2).sum())


def _repair_bins(bins, order, starts, idx_of_inc, cap=128, iters=400):
    """Greedy rebalance: move members out of the window with the most
    post-dedup slots into the one with the fewest (respecting the member
    cap), to minimize max slots per window."""
    srcs = [
        [idx_of_inc[order[starts[d]:starts[d + 1]]] for d in b] for b in bins
    ]

    def slots(w):
        return _dedup_slots(np.concatenate(srcs[w]) if srcs[w] else
                            np.zeros(0, np.int64))

    cur = [slots(w) for w in range(len(bins))]
    for _ in range(iters):
        hot = int(np.argmax(cur))
        order_cold = np.argsort(cur)
        moved = False
        for cold in order_cold:
            if cold == hot or len(bins[cold]) >= cap:
                continue
            # move the member with the smallest segment out of `hot`
            j = int(np.argmin([len(s) for s in srcs[hot]]))
            bins[cold].append(bins[hot].pop(j))
            srcs[cold].append(srcs[hot].pop(j))
            new_hot, new_cold = slots(hot), slots(cold)
            if max(new_hot, new_cold) >= cur[hot]:
                # revert: no improvement
                bins[hot].append(bins[cold].pop())
                srcs[hot].append(srcs[cold].pop())
                continue
            cur[hot], cur[cold] = new_hot, new_cold
            moved = True
            break
        if not moved:
            break
    return bins


def make_plan(v_idx, e_idx, e2v_weight, nv=NV, ne=NE, ch=CH, ncores=NCORES):
    P = Plan()
    P.nv, P.ne, P.ch, P.ncores = nv, ne, ch, ncores
    epc, vpc = ne // ncores, nv // ncores
    P.epc, P.vpc = epc, vpc

    order_e, deg_e, starts_e = _csr(e_idx, ne)
    order_v, deg_v, starts_v = _csr(v_idx, nv)
    inv_deg = np.zeros(ne, np.float32)
    nz = deg_e > 0
    inv_deg[nz] = (np.float32(1.0) / deg_e[nz].astype(np.float32))

    nb1 = -(-epc // 128)
    nb2 = -(-vpc // 128)
    v_of_inc = v_idx.astype(np.int64)
    e_of_inc = e_idx.astype(np.int64)
    # balance destinations across cores globally (assignment is free — pos /
    # vmap carry it), then binpack windows within each core and rebalance for
    # post-dedup slot counts.
    cores_e, _ = _binpack(np.arange(ne), deg_e, ncores, cap=nb1 * 128)
    bins1 = []
    for k in range(ncores):
        eids = np.asarray(cores_e[k])
        b, _ = _binpack(eids, deg_e[eids], nb1)
        bins1.append(_repair_bins(b, order_e, starts_e, v_of_inc))

    # p2 window count: an extra window can admit a smaller W2 (less gather
    # padding) once dedup shrinks the per-window loads — pick the best.
    best = None
    for nb2c in (nb2, nb2 + 1):
        cores_v, _ = _binpack(np.arange(nv), deg_v, ncores, cap=nb2c * 128)
        cand = []
        wmax = 0
        for k in range(ncores):
            vids = np.asarray(cores_v[k])
            b, _ = _binpack(vids, deg_v[vids], nb2c)
            b = _repair_bins(b, order_v, starts_v, e_of_inc)
            cand.append(b)
            for bb in b:
                wmax = max(wmax, _dedup_slots(np.concatenate(
                    [e_of_inc[order_v[starts_v[d]:starts_v[d + 1]]]
                     for d in bb]) if bb else np.zeros(0, np.int64)))
        W2c = -(-wmax // 128)
        if best is None or nb2c * W2c < best[0] * best[1]:
            best = (nb2c, W2c, cand)
    nb2, _, bins2 = best
    P.NW1, P.NW2 = nb1, nb2

    # phase-1 windows (dedup within window) + edge position map.  Positions
    # are window-major (w, core, row) so each window's table slice can be
    # AllGathered independently as soon as its GEMM finishes.
    pos = np.zeros(ne, np.int64)
    wins1 = []
    w1max = 0
    for k in range(ncores):
        wins, wmax = _phase_windows(
            bins1[k], order_e, starts_e, v_idx.astype(np.int64),
            inv_deg[e_idx.astype(np.int64)])
        wins1.append(wins)
        w1max = max(w1max, wmax)
        for w, bin_ids in enumerate(bins1[k]):
            for j, e in enumerate(bin_ids):
                if AG_SLICED:   # window-major: (w, core, row)
                    pos[e] = (w * ncores + k) * 128 + j
                else:           # core-major: AllGather rank concatenation
                    pos[e] = (k * nb1 + w) * 128 + j
    assert pos.max() < 32768
    P.W1 = -(-w1max // 128)
    P.C1 = P.NW1 * P.W1
    P.p1 = [_layout(wins, P.W1, P.NW1) for wins in wins1]

    # phase-2 windows + output row map. Softmax weights are fully normalized
    # on the host (exp / per-vertex sum), so the kernel needs no denominator
    # pass.
    expw = np.exp(e2v_weight.astype(np.float64))
    den = np.zeros(nv, np.float64)
    np.add.at(den, v_idx, expw)
    wnorm = (expw / den[v_idx.astype(np.int64)]).astype(np.float32)
    wins2 = []
    w2max = 0
    P.vmap = []
    for k in range(ncores):
        wins, wmax = _phase_windows(
            bins2[k], order_v, starts_v, pos[e_idx.astype(np.int64)], wnorm)
        wins2.append(wins)
        w2max = max(w2max, wmax)
        vm = np.full(P.NW2 * 128, -1, np.int64)
        for w, bin_ids in enumerate(bins2[k]):
            vm[w * 128:w * 128 + len(bin_ids)] = bin_ids
        # deg-0 vertices never receive contributions; drop them from the
        # output map so any on-chip garbage (e.g. a stray +b) is discarded.
        vme = vm[vm >= 0]
        vm[vm >= 0] = np.where(deg_v[vme] > 0, vme, -1)
        P.vmap.append(vm)
    P.W2 = -(-w2max // 128)
    P.C2 = P.NW2 * P.W2
    P.p2 = [_layout(wins, P.W2, P.NW2) for wins in wins2]
    return P


# ---------------------------------------------------------------- builder ---
def build_nc(P, bf16=GATHER_BF16, spmd=True, reps=1, grp=GRP, gbufs=6,
             nqueues=1, p1_dt=P1_DT, p2_dt=P2_DT, out_dt=OUT_DT, fuse=FUSE):
    import concourse.bacc as bacc
    import concourse.mybir as mybir
    import concourse.tile as tile

    f32 = mybir.dt.float32
    dt_g = mybir.dt.bfloat16 if bf16 else f32   # sel matrices + GEMM operands
    dt_p1 = _mydt(mybir, p1_dt)
    dt_p2 = _mydt(mybir, p2_dt)
    dt_out = _mydt(mybir, out_dt)
    eq, mul, mx, add = (mybir.AluOpType.is_equal, mybir.AluOpType.mult,
                        mybir.AluOpType.max, mybir.AluOpType.add)
    ch, KT = P.ch, P.ch // 128
    post = fuse == "post"

    nc = bacc.Bacc("TRN2", target_bir_lowering=False, debug=False,
                   num_devices=P.ncores if spmd else 1,
                   num_swdge_queues=nqueues)

    XT = nc.dram_tensor("xt", [P.nv, ch], dt_p1, kind="ExternalInput")
    WT = nc.dram_tensor("wt", [128, KT, ch], dt_g, kind="ExternalInput")
    BT = nc.dram_tensor("bt", [1, ch], dt_g, kind="ExternalInput")
    IOTA = nc.dram_tensor("iota", [128, 128], dt_g, kind="ExternalInput")
    IDENT = nc.dram_tensor("ident", [128, 128], dt_g, kind="ExternalInput")
    G1IDX = nc.dram_tensor("g1idx", [128, P.C1 * 8], mybir.dt.int16, kind="ExternalInput")
    ELOC1 = nc.dram_tensor("eloc1", [128, P.C1], f32, kind="ExternalInput")
    WSEL1 = nc.dram_tensor("wsel1", [128, P.C1], f32, kind="ExternalInput")
    ELOC1B = nc.dram_tensor("eloc1b", [128, P.C1], f32, kind="ExternalInput")
    WSEL1B = nc.dram_tensor("wsel1b", [128, P.C1], f32, kind="ExternalInput")
    G2IDX = nc.dram_tensor("g2idx", [128, P.C2 * 8], mybir.dt.int16, kind="ExternalInput")
    VLOC2 = nc.dram_tensor("vloc2", [128, P.C2], f32, kind="ExternalInput")
    W2RAW = nc.dram_tensor("w2raw", [128, P.C2], f32, kind="ExternalInput")
    VLOC2B = nc.dram_tensor("vloc2b", [128, P.C2], f32, kind="ExternalInput")
    W2RAWB = nc.dram_tensor("w2rawb", [128, P.C2], f32, kind="ExternalInput")

    ner1 = P.NW1 * 128
    CCIN = nc.dram_tensor("ccin", [ner1, ch], dt_p2)
    CCOUT = nc.dram_tensor("ccout", [P.ncores * ner1, ch], dt_p2, addr_space="Shared")
    OUT = nc.dram_tensor("out", [P.NW2 * 128, ch], dt_out, kind="ExternalOutput")

    with tile.TileContext(nc) as tc, ExitStack() as ctx:
        const = ctx.enter_context(tc.tile_pool(name="const", bufs=1))
        gpool = ctx.enter_context(tc.tile_pool(name="g", bufs=gbufs))
        selp = ctx.enter_context(tc.tile_pool(name="selp", bufs=6))
        psum = ctx.enter_context(tc.tile_pool(name="ps", bufs=2, space="PSUM"))
        sbp = ctx.enter_context(tc.tile_pool(name="sbp", bufs=2))

        def cload(dram, shape, dt, tag):
            t = const.tile(shape, dt, tag=tag)
            nc.sync.dma_start(t[:], dram[:])
            return t

        # p1-critical tables first: the first gather waits only on these.
        iota_t = cload(IOTA, [128, 128], dt_g, "iota")
        g1idx_t = cload(G1IDX, [128, P.C1 * 8], mybir.dt.int16, "g1idx")
        eloc1_t = cload(ELOC1, [128, P.C1], f32, "eloc1")
        wsel1_t = cload(WSEL1, [128, P.C1], f32, "wsel1")
        eloc1b_t = cload(ELOC1B, [128, P.C1], f32, "eloc1b")
        wsel1b_t = cload(WSEL1B, [128, P.C1], f32, "wsel1b")
        wt_t = cload(WT, [128, KT, ch], dt_g, "wt")
        bt_t = cload(BT, [1, ch], dt_g, "bt")
        ident_t = cload(IDENT, [128, 128], dt_g, "ident")
        g2idx_t = cload(G2IDX, [128, P.C2 * 8], mybir.dt.int16, "g2idx")
        vloc2_t = cload(VLOC2, [128, P.C2], f32, "vloc2")
        w2raw_t = cload(W2RAW, [128, P.C2], f32, "w2raw")
        vloc2b_t = cload(VLOC2B, [128, P.C2], f32, "vloc2b")
        w2rawb_t = cload(W2RAWB, [128, P.C2], f32, "w2rawb")
        ones1_t = const.tile([1, 128], dt_g, tag="ones1")
        nc.vector.memset(ones1_t[:], 1.0)

        # ---------------- gather + one/two-hot reduce ----------------------
        def agg_phase(src_ap, gidx_t, loc_t, w_t, locb_t, wb_t, is2, C, W,
                      gtag, chunk_cb, win_cb, dt_tab):
            pw = None
            for g0 in range(0, C, grp):
                n = min(grp, C - g0)
                gt = gpool.tile([128, n, ch], dt_tab, tag=gtag)
                nc.gpsimd.dma_gather(
                    gt[:], src_ap, gidx_t[:, g0 * 8:(g0 + n) * 8],
                    n * 128, n * 128, ch, queue_num=(g0 // grp) % nqueues)
                for j in range(n):
                    c = g0 + j
                    w, cw = divmod(c, W)
                    sel = selp.tile([128, 128], dt_g, tag="sel")
                    nc.vector.tensor_scalar(
                        sel[:], iota_t[:], loc_t[:, c:c + 1], w_t[:, c:c + 1],
                        op0=eq, op1=mul)
                    if is2[c]:  # dedup chunk: add the second hot
                        selb = selp.tile([128, 128], dt_g, tag="selb")
                        nc.vector.tensor_scalar(
                            selb[:], iota_t[:], locb_t[:, c:c + 1],
                            wb_t[:, c:c + 1], op0=eq, op1=mul)
                        sel2 = selp.tile([128, 128], dt_g, tag="sel2")
                        nc.vector.tensor_tensor(sel2[:], sel[:], selb[:], op=add)
                        sel = sel2
                    if cw == 0:
                        pw = psum.tile([128, ch], f32, tag="win")
                    chunk_cb(pw, sel, gt, j, w, cw, cw == W - 1)
                    if cw == W - 1:
                        win_cb(pw, w)

        def p1_chunk(pw, sel, gt, j, w, cw, last):
            nc.tensor.matmul(pw[:], sel[:], gt[:, j, :],
                             start=(cw == 0), stop=last)

        def gemm_bias(src_t, dst_psum):
            """dst[v/e, co] = src^T blocks @ W.T + 1^T b (K=1 bias matmul)."""
            for k in range(KT):
                nc.tensor.matmul(dst_psum[:], src_t[:, k, :], wt_t[:, k, :],
                                 start=(k == 0), stop=False)
            nc.tensor.matmul(dst_psum[:], ones1_t[:], bt_t[:],
                             start=False, stop=True)

        def transpose_blocks(pw, tag):
            """psum [128, ch] f32 -> sbuf [128, KT, 128] dt_g transposed."""
            t_w = sbp.tile([128, ch], dt_g, tag=tag + "f", name=tag + "f")
            nc.vector.tensor_copy(t_w[:], pw[:])
            tT_w = sbp.tile([128, KT, 128], dt_g, tag=tag + "T", name=tag + "T")
            for k in range(KT):
                pt = psum.tile([128, 128], dt_g, tag="aux", name="pt")
                nc.tensor.transpose(pt[:], t_w[:, k * 128:(k + 1) * 128],
                                    ident_t[:])
                nc.vector.tensor_copy(tT_w[:, k, :], pt[:])
            return tT_w

        def p1_win(pw, w):
            # window w's edge rows are complete: ship its CCIN slice and
            # immediately AllGather that window's table piece, overlapping
            # the collective with the remaining p1 windows.
            xep = sbp.tile([128, ch], dt_p2, tag="xep", name="xep")
            if post:
                # raw table, scaled for fp8 range; GEMM happens after p2 agg
                nc.vector.tensor_scalar(xep[:], pw[:], float(P2_SCALE), None,
                                        op0=mul)
            else:
                xeT_w = transpose_blocks(pw, "xe")
                pg = psum.tile([128, ch], f32, tag="gemm", name="pg")
                gemm_bias(xeT_w, pg)
                nc.vector.tensor_copy(xep[:], pg[:])
            nc.sync.dma_start(CCIN[w * 128:(w + 1) * 128, :], xep[:])
            if AG_SLICED:
                o = w * P.ncores * 128
                if spmd:
                    nc.gpsimd.collective_compute(
                        "AllGather", mybir.AluOpType.bypass,
                        replica_groups=[list(range(P.ncores))],
                        ins=[CCIN[w * 128:(w + 1) * 128, :]],
                        outs=[CCOUT[o:o + P.ncores * 128, :]])
                else:  # single-core stand-in for the window AllGather
                    nc.sync.dma_start(CCOUT[o:o + 128, :],
                                      CCIN[w * 128:(w + 1) * 128, :])
            elif w == P.NW1 - 1:
                if spmd:
                    nc.gpsimd.collective_compute(
                        "AllGather", mybir.AluOpType.bypass,
                        replica_groups=[list(range(P.ncores))],
                        ins=[CCIN[:]], outs=[CCOUT[:]])
                else:
                    nc.sync.dma_start(CCOUT[0:ner1, :], CCIN[:])

        def p2_chunk(pw, sel, gt, j, w, cw, last):
            nc.tensor.matmul(pw[:], sel[:], gt[:, j, :],
                             start=(cw == 0), stop=last)

        def p2_win(pw, w):
            if post:
                awT = transpose_blocks(pw, "aw")
                po = psum.tile([128, ch], f32, tag="gemm", name="po")
                gemm_bias(awT, po)
                pw = po
            # weights pre-normalized on host: just relu + store
            ow = sbp.tile([128, ch], dt_out, tag="ow", name="ow")
            nc.vector.tensor_scalar(ow[:], pw[:], 1.0, 0.0, op0=mul, op1=mx)
            nc.sync.dma_start(OUT[w * 128:(w + 1) * 128, :], ow[:])

        # chunks that contain any dedup pair need the second sel pass; the
        # union over cores keeps the SPMD program identical on every core.
        is2_1 = np.zeros(P.C1, bool)
        is2_2 = np.zeros(P.C2, bool)
        for k in range(P.ncores):
            is2_1 |= (_pack(P.p1[k][3], P.C1) >= 0).any(axis=0)
            is2_2 |= (_pack(P.p2[k][3], P.C2) >= 0).any(axis=0)

        for _rep in range(reps):
            agg_phase(XT[:], g1idx_t, eloc1_t, wsel1_t, eloc1b_t, wsel1b_t,
                      is2_1, P.C1, P.W1, "g1", p1_chunk, p1_win, dt_p1)

            # phase 2: e2v aggregation (sel weights pre-normalized on host)
            agg_phase(CCOUT[:], g2idx_t, vloc2_t, w2raw_t, vloc2b_t, w2rawb_t,
                      is2_2, P.C2, P.W2, "g2", p2_chunk, p2_win, dt_p2)

    nc.compile()
    return nc


# ------------------------------------------------------------------ runner ---
def make_in_maps(P, X, W, b, bf16=GATHER_BF16, p1_dt=P1_DT, fuse=FUSE):
    npdt = ml_dtypes.bfloat16 if bf16 else np.float32
    np_p1 = _npdt(p1_dt)
    s1 = P1_SCALE if p1_dt == "f8" else 1.0
    s2 = P2_SCALE if fuse == "post" else 1.0
    KT = P.ch // 128
    xt = np.ascontiguousarray((X * s1).astype(np_p1))
    wt = np.ascontiguousarray(
        W.T.reshape(KT, 128, P.ch).transpose(1, 0, 2).astype(npdt))
    bt = np.ascontiguousarray(b.astype(npdt).reshape(1, P.ch))
    iota = np.ascontiguousarray(
        np.broadcast_to(np.arange(128, dtype=npdt), (128, 128)))
    ident = np.eye(128, dtype=npdt)

    def tb(flat, C, s=1.0):
        return _pack(flat, C) / np.float32(s)

    in_maps = []
    for k in range(P.ncores):
        g1, l1, w1, l1b, w1b = P.p1[k]
        g2, l2, w2, l2b, w2b = P.p2[k]
        in_maps.append({
            "xt": xt, "wt": wt, "bt": bt, "iota": iota, "ident": ident,
            "g1idx": _wrap_idx(g1), "eloc1": tb(l1, P.C1),
            "wsel1": tb(w1, P.C1, s1), "eloc1b": tb(l1b, P.C1),
            "wsel1b": tb(w1b, P.C1, s1),
            "g2idx": _wrap_idx(g2), "vloc2": tb(l2, P.C2),
            "w2raw": tb(w2, P.C2, s2), "vloc2b": tb(l2b, P.C2),
            "w2rawb": tb(w2b, P.C2, s2),
        })
    return in_maps


def assemble(P, shards):
    out = np.zeros((P.nv, P.ch), np.float32)
    for k in range(P.ncores):
        vm = P.vmap[k]
        m = vm >= 0
        out[vm[m]] = shards[k][m].astype(np.float32)
    return out


_nc_cache = {}


def kernel(X, W, b, e2v_weight, v_idx, e_idx):
    global _last_results
    from concourse.bass_utils import run_bass_kernel_spmd

    P = make_plan(v_idx, e_idx, e2v_weight)
    key = (P.C1, P.C2, P.W1, P.W2, GATHER_BF16, P1_DT, P2_DT, OUT_DT, FUSE)
    if key not in _nc_cache:
        _nc_cache[key] = build_nc(P)
    nc = _nc_cache[key]
    in_maps = make_in_maps(P, X, W, b)
    res = run_bass_kernel_spmd(nc, in_maps, list(range(P.ncores)), trace=TRACE)
    _last_results = res
    shards = [res.results[k]["out"] for k in range(P.ncores)]
    return assemble(P, shards)



# revision 59
# speedup vs baseline: 1.0472x; 1.0059x over previous
"""HGNNPConv Trainium2 kernel (8 NeuronCores, SPMD).

Math (equivalent reformulation of the reference):
  Xe_raw[e] = mean_{i: e_idx[i]=e} X[v_idx[i]]              (v2e, softmax of ones = 1/deg)
  Xe_p      = Xe_raw @ W.T + b                              (GEMM on 4000 edges, not 20000 verts)
  Xv[v]     = sum_i wn_i * Xe_p[e_idx[i]],  wn_i = exp(w_i)/sum_{v} exp(w)
              (wn precomputed on host -> no on-chip denominator pass)
  out       = relu(Xv)
Empty edges get a spurious +b in Xe_p but are never referenced downstream
(an edge appearing in phase 2 has >=1 incidence, hence deg>=1 in phase 1).

Sharding: phase 1 by destination edge (500/core), edge-level GEMM per core,
AllGather of the projected edge table (1MB/core), phase 2 by destination
vertex (2500/core). Per-destination-window weighted one-hot selection
matrices (built on DVE from iota) reduce gathered rows on the PE into PSUM.
Phase-1 gather table is fp8-e3m4 (X pre-scaled by 2), phase-2 table bf16,
output bf16 (upcast on host).
"""

import os
from contextlib import ExitStack

import numpy as np
import ml_dtypes

# ---------------------------------------------------------------- config ---
NCORES = 8
NV, NE, NNZ, CH = 20000, 4000, 160000, 512
GATHER_BF16 = os.environ.get("KERNEL_F32", "") == ""  # bf16 tables+matmuls by default
P1_DT = os.environ.get("KERNEL_P1_DT", "f8")   # phase-1 gather table dtype
P2_DT = os.environ.get("KERNEL_P2_DT", "bf16")  # phase-2 gather table dtype
OUT_DT = os.environ.get("KERNEL_OUT_DT", "bf16")
FUSE = os.environ.get("KERNEL_FUSE", "pre")    # "post": GEMM after p2 agg
P1_SCALE = 2.0   # X pre-scale for fp8-e3m4 range use (exact power of 2)
P2_SCALE = 8.0   # Xe_raw pre-scale for the fp8 edge table (post mode)
GRP = 5          # gather chunks (of 128 idxs) per dma_gather call
AG_SLICED = os.environ.get("KERNEL_AG_SLICED", "") != ""  # per-window AllGather
TRACE = os.environ.get("BASS_TRACE", "") != ""


def _mydt(mybir, name):
    return {"f8": mybir.dt.float8e3, "bf16": mybir.dt.bfloat16,
            "f32": mybir.dt.float32}[name]


def _npdt(name):
    return {"f8": ml_dtypes.float8_e3m4, "bf16": ml_dtypes.bfloat16,
            "f32": np.float32}[name]

_last_results = None   # BassKernelResults of the most recent run (for test.py)


# ------------------------------------------------------------------- plan ---
class Plan:
    pass


def _binpack(ids, degs, nbins, cap=128):
    """Pack `ids` into `nbins` bins of <=cap items, balancing sum(degs)."""
    import heapq

    order = np.argsort(-degs, kind="stable")
    bins = [[] for _ in range(nbins)]
    loads = [0] * nbins
    heap = [(0, b) for b in range(nbins)]
    heapq.heapify(heap)
    for t in order:
        popped = []
        while True:
            load, b = heapq.heappop(heap)
            if len(bins[b]) < cap:
                break
            popped.append((load, b))
        for p in popped:
            heapq.heappush(heap, p)
        bins[b].append(int(ids[t]))
        loads[b] = load + int(degs[t])
        heapq.heappush(heap, (loads[b], b))
    return bins, loads


def _csr(idx, n):
    order = np.argsort(idx, kind="stable").astype(np.int64)
    deg = np.bincount(idx, minlength=n).astype(np.int64)
    starts = np.zeros(n + 1, np.int64)
    np.cumsum(deg, out=starts[1:])
    return order, deg, starts


def _pair_window(src, locs, ws):
    """Greedy within-window dedup: incidences sharing a source row become one
    gathered slot with two (loc, w) hots.  Returns (src', loc_a, w_a, loc_b,
    w_b) with pair slots FIRST; singles have loc_b = -1 / w_b = 0."""
    order = np.argsort(src, kind="stable")
    src, locs, ws = src[order], locs[order], ws[order]
    pa, pb, sg = [], [], []
    i, n = 0, len(src)
    while i < n:
        j = i
        while j < n and src[j] == src[i]:
            j += 1
        k = i
        while k + 1 < j:
            pa.append(k); pb.append(k + 1); k += 2
        if k < j:
            sg.append(k)
        i = j
    pa, pb, sg = np.array(pa, np.int64), np.array(pb, np.int64), np.array(sg, np.int64)
    src2 = np.concatenate([src[pa], src[sg]]) if len(pa) else src[sg]
    la = np.concatenate([locs[pa], locs[sg]]) if len(pa) else locs[sg]
    wa = np.concatenate([ws[pa], ws[sg]]) if len(pa) else ws[sg]
    lb = np.concatenate([locs[pb], np.full(len(sg), -1.0, locs.dtype)]) if len(pa) \
        else np.full(len(sg), -1.0, locs.dtype)
    wb = np.concatenate([ws[pb], np.zeros(len(sg), ws.dtype)]) if len(pa) \
        else np.zeros(len(sg), ws.dtype)
    return src2, la, wa, lb, wb, len(pa)


def _phase_windows(bins_per_core, order, starts, idx_of_inc, w_of_inc,
                   loc_dtype=np.float32, pair=True):
    """Per-window slot lists for one core of one phase, after source dedup.

    Returns (wins, wmax): wins[w] = (src, loc_a, w_a, loc_b, w_b); pair slots
    (loc_b >= 0) come first within each window.
    """
    wins = []
    wmax = 0
    for bin_ids in bins_per_core:
        incs = []
        locs = []
        for j, d in enumerate(bin_ids):
            seg = order[starts[d]:starts[d + 1]]
            incs.append(seg)
            locs.append(np.full(len(seg), j, loc_dtype))
        incs = np.concatenate(incs) if incs else np.zeros(0, np.int64)
        locs = np.concatenate(locs) if locs else np.zeros(0, loc_dtype)
        src = idx_of_inc[incs]
        ws = w_of_inc[incs].astype(np.float32)
        if pair and len(src):
            src, la, wa, lb, wb, _ = _pair_window(src, locs, ws)
        else:
            la, wa = locs, ws
            lb = np.full(len(src), -1.0, loc_dtype)
            wb = np.zeros(len(src), np.float32)
        wins.append((src, la, wa, lb, wb))
        wmax = max(wmax, len(src))
    return wins, wmax


def _layout(wins, W, nw):
    """Flat slot arrays: slot i = (chunk i//128, partition i%128); chunk c
    belongs to window c//W."""
    L = nw * W * 128
    gidx = np.zeros(L, np.int16)
    loc = np.full(L, -1.0, np.float32)
    wsel = np.zeros(L, np.float32)
    locb = np.full(L, -1.0, np.float32)
    wselb = np.zeros(L, np.float32)
    for w, (src, la, wa, lb, wb) in enumerate(wins):
        n = len(src)
        o = w * W * 128
        gidx[o:o + n] = src
        loc[o:o + n] = la
        wsel[o:o + n] = wa
        locb[o:o + n] = lb
        wselb[o:o + n] = wb
    return gidx, loc, wsel, locb, wselb


def _wrap_idx(flat):
    """int16 flat[i] -> [128, len/16] with value i at [i%16, i//16], replicated."""
    a = flat.reshape(-1, 16).T  # [16, L/16]
    return np.ascontiguousarray(np.tile(a, (8, 1)))


def _pack(flat, C):
    """flat[c*128+p] -> [128, C]"""
    return np.ascontiguousarray(flat.reshape(C, 128).T)


def _dedup_slots(srcs):
    """#gather slots for a window's source list after pairing."""
    if not len(srcs):
        return 0
    _, cnt = np.unique(srcs, return_counts=True)
    return int(((cnt + 1) // 2).sum())


def _repair_bins(bins, order, starts, idx_of_inc, cap=128, iters=400):
    """Greedy rebalance: move members out of the window with the most
    post-dedup slots into the one with the fewest (respecting the member
    cap), to minimize max slots per window."""
    srcs = [
        [idx_of_inc[order[starts[d]:starts[d + 1]]] for d in b] for b in bins
    ]

    def slots(w):
        return _dedup_slots(np.concatenate(srcs[w]) if srcs[w] else
                            np.zeros(0, np.int64))

    cur = [slots(w) for w in range(len(bins))]
    for _ in range(iters):
        hot = int(np.argmax(cur))
        order_cold = np.argsort(cur)
        moved = False
        for cold in order_cold:
            if cold == hot or len(bins[cold]) >= cap:
                continue
            # move the member with the smallest segment out of `hot`
            j = int(np.argmin([len(s) for s in srcs[hot]]))
            bins[cold].append(bins[hot].pop(j))
            srcs[cold].append(srcs[hot].pop(j))
            new_hot, new_cold = slots(hot), slots(cold)
            if max(new_hot, new_cold) >= cur[hot]:
                # revert: no improvement
                bins[hot].append(bins[cold].pop())
                srcs[hot].append(srcs[cold].pop())
                continue
            cur[hot], cur[cold] = new_hot, new_cold
            moved = True
            break
        if not moved:
            break
    return bins


def make_plan(v_idx, e_idx, e2v_weight, nv=NV, ne=NE, ch=CH, ncores=NCORES):
    P = Plan()
    P.nv, P.ne, P.ch, P.ncores = nv, ne, ch, ncores
    epc, vpc = ne // ncores, nv // ncores
    P.epc, P.vpc = epc, vpc

    order_e, deg_e, starts_e = _csr(e_idx, ne)
    order_v, deg_v, starts_v = _csr(v_idx, nv)
    inv_deg = np.zeros(ne, np.float32)
    nz = deg_e > 0
    inv_deg[nz] = (np.float32(1.0) / deg_e[nz].astype(np.float32))

    nb1 = -(-epc // 128)
    nb2 = -(-vpc // 128)
    v_of_inc = v_idx.astype(np.int64)
    e_of_inc = e_idx.astype(np.int64)
    # balance destinations across cores globally (assignment is free — pos /
    # vmap carry it), then binpack windows within each core and rebalance for
    # post-dedup slot counts.
    cores_e, _ = _binpack(np.arange(ne), deg_e, ncores, cap=nb1 * 128)
    bins1 = []
    for k in range(ncores):
        eids = np.asarray(cores_e[k])
        b, _ = _binpack(eids, deg_e[eids], nb1)
        bins1.append(_repair_bins(b, order_e, starts_e, v_of_inc))

    # p2 window count: an extra window can admit a smaller W2 (less gather
    # padding) once dedup shrinks the per-window loads — pick the best.
    best = None
    for nb2c in (nb2, nb2 + 1):
        cores_v, _ = _binpack(np.arange(nv), deg_v, ncores, cap=nb2c * 128)
        cand = []
        wmax = 0
        for k in range(ncores):
            vids = np.asarray(cores_v[k])
            b, _ = _binpack(vids, deg_v[vids], nb2c)
            b = _repair_bins(b, order_v, starts_v, e_of_inc)
            cand.append(b)
            for bb in b:
                wmax = max(wmax, _dedup_slots(np.concatenate(
                    [e_of_inc[order_v[starts_v[d]:starts_v[d + 1]]]
                     for d in bb]) if bb else np.zeros(0, np.int64)))
        W2c = -(-wmax // 128)
        if best is None or nb2c * W2c < best[0] * best[1]:
            best = (nb2c, W2c, cand)
    nb2, _, bins2 = best
    P.NW1, P.NW2 = nb1, nb2

    # phase-1 windows (dedup within window) + edge position map.  Positions
    # are window-major (w, core, row) so each window's table slice can be
    # AllGathered independently as soon as its GEMM finishes.
    pos = np.zeros(ne, np.int64)
    wins1 = []
    w1max = 0
    for k in range(ncores):
        wins, wmax = _phase_windows(
            bins1[k], order_e, starts_e, v_idx.astype(np.int64),
            inv_deg[e_idx.astype(np.int64)])
        wins1.append(wins)
        w1max = max(w1max, wmax)
        for w, bin_ids in enumerate(bins1[k]):
            for j, e in enumerate(bin_ids):
                if AG_SLICED:   # window-major: (w, core, row)
                    pos[e] = (w * ncores + k) * 128 + j
                else:           # core-major: AllGather rank concatenation
                    pos[e] = (k * nb1 + w) * 128 + j
    assert pos.max() < 32768
    P.W1 = -(-w1max // 128)
    P.C1 = P.NW1 * P.W1
    P.p1 = [_layout(wins, P.W1, P.NW1) for wins in wins1]

    # phase-2 windows + output row map. Softmax weights are fully normalized
    # on the host (exp / per-vertex sum), so the kernel needs no denominator
    # pass.
    expw = np.exp(e2v_weight.astype(np.float64))
    den = np.zeros(nv, np.float64)
    np.add.at(den, v_idx, expw)
    wnorm = (expw / den[v_idx.astype(np.int64)]).astype(np.float32)
    wins2 = []
    w2max = 0
    P.vmap = []
    for k in range(ncores):
        wins, wmax = _phase_windows(
            bins2[k], order_v, starts_v, pos[e_idx.astype(np.int64)], wnorm)
        wins2.append(wins)
        w2max = max(w2max, wmax)
        vm = np.full(P.NW2 * 128, -1, np.int64)
        for w, bin_ids in enumerate(bins2[k]):
            vm[w * 128:w * 128 + len(bin_ids)] = bin_ids
        # deg-0 vertices never receive contributions; drop them from the
        # output map so any on-chip garbage (e.g. a stray +b) is discarded.
        vme = vm[vm >= 0]
        vm[vm >= 0] = np.where(deg_v[vme] > 0, vme, -1)
        P.vmap.append(vm)
    P.W2 = -(-w2max // 128)
    P.C2 = P.NW2 * P.W2
    P.p2 = [_layout(wins, P.W2, P.NW2) for wins in wins2]
    return P


# ---------------------------------------------------------------- builder ---
def build_nc(P, bf16=GATHER_BF16, spmd=True, reps=1, grp=GRP, gbufs=6,
             nqueues=1, p1_dt=P1_DT, p2_dt=P2_DT, out_dt=OUT_DT, fuse=FUSE):
    import concourse.bacc as bacc
    import concourse.mybir as mybir
    import concourse.tile as tile

    f32 = mybir.dt.float32
    dt_g = mybir.dt.bfloat16 if bf16 else f32   # sel matrices + GEMM operands
    dt_p1 = _mydt(mybir, p1_dt)
    dt_p2 = _mydt(mybir, p2_dt)
    dt_out = _mydt(mybir, out_dt)
    eq, mul, mx, add = (mybir.AluOpType.is_equal, mybir.AluOpType.mult,
                        mybir.AluOpType.max, mybir.AluOpType.add)
    ch, KT = P.ch, P.ch // 128
    post = fuse == "post"

    nc = bacc.Bacc("TRN2", target_bir_lowering=False, debug=False,
                   num_devices=P.ncores if spmd else 1,
                   num_swdge_queues=nqueues)

    XT = nc.dram_tensor("xt", [P.nv, ch], dt_p1, kind="ExternalInput")
    WT = nc.dram_tensor("wt", [128, KT, ch], dt_g, kind="ExternalInput")
    BT = nc.dram_tensor("bt", [1, ch], dt_g, kind="ExternalInput")
    IOTA = nc.dram_tensor("iota", [128, 128], dt_g, kind="ExternalInput")
    IDENT = nc.dram_tensor("ident", [128, 128], dt_g, kind="ExternalInput")
    G1IDX = nc.dram_tensor("g1idx", [128, P.C1 * 8], mybir.dt.int16, kind="ExternalInput")
    P1TAB = nc.dram_tensor("p1tab", [128, 4, P.C1], f32, kind="ExternalInput")
    G2IDX = nc.dram_tensor("g2idx", [128, P.C2 * 8], mybir.dt.int16, kind="ExternalInput")
    P2TAB = nc.dram_tensor("p2tab", [128, 4, P.C2], f32, kind="ExternalInput")

    ner1 = P.NW1 * 128
    CCIN = nc.dram_tensor("ccin", [ner1, ch], dt_p2)
    CCOUT = nc.dram_tensor("ccout", [P.ncores * ner1, ch], dt_p2, addr_space="Shared")
    OUT = nc.dram_tensor("out", [P.NW2 * 128, ch], dt_out, kind="ExternalOutput")

    with tile.TileContext(nc) as tc, ExitStack() as ctx:
        const = ctx.enter_context(tc.tile_pool(name="const", bufs=1))
        gpool = ctx.enter_context(tc.tile_pool(name="g", bufs=gbufs))
        selp = ctx.enter_context(tc.tile_pool(name="selp", bufs=6))
        psum = ctx.enter_context(tc.tile_pool(name="ps", bufs=2, space="PSUM"))
        sbp = ctx.enter_context(tc.tile_pool(name="sbp", bufs=2))

        def cload(dram, shape, dt, tag, eng=None):
            t = const.tile(shape, dt, tag=tag)
            (eng or nc.sync).dma_start(t[:], dram[:])
            return t

        # p1-critical tables first (SP ring); the rest go on the ACT ring so
        # they never delay the first gather.
        iota_t = cload(IOTA, [128, 128], dt_g, "iota")
        g1idx_t = cload(G1IDX, [128, P.C1 * 8], mybir.dt.int16, "g1idx")
        p1tab_t = cload(P1TAB, [128, 4, P.C1], f32, "p1tab")
        eloc1_t, wsel1_t = p1tab_t[:, 0, :], p1tab_t[:, 1, :]
        eloc1b_t, wsel1b_t = p1tab_t[:, 2, :], p1tab_t[:, 3, :]
        wt_t = cload(WT, [128, KT, ch], dt_g, "wt", eng=nc.scalar)
        bt_t = cload(BT, [1, ch], dt_g, "bt", eng=nc.scalar)
        ident_t = cload(IDENT, [128, 128], dt_g, "ident", eng=nc.scalar)
        g2idx_t = cload(G2IDX, [128, P.C2 * 8], mybir.dt.int16, "g2idx",
                        eng=nc.scalar)
        p2tab_t = cload(P2TAB, [128, 4, P.C2], f32, "p2tab", eng=nc.scalar)
        vloc2_t, w2raw_t = p2tab_t[:, 0, :], p2tab_t[:, 1, :]
        vloc2b_t, w2rawb_t = p2tab_t[:, 2, :], p2tab_t[:, 3, :]
        ones1_t = const.tile([1, 128], dt_g, tag="ones1")
        nc.vector.memset(ones1_t[:], 1.0)

        # ---------------- gather + one/two-hot reduce ----------------------
        def agg_phase(src_ap, gidx_t, loc_t, w_t, locb_t, wb_t, is2, C, W,
                      gtag, chunk_cb, win_cb, dt_tab):
            pw = None
            for g0 in range(0, C, grp):
                n = min(grp, C - g0)
                gt = gpool.tile([128, n, ch], dt_tab, tag=gtag)
                nc.gpsimd.dma_gather(
                    gt[:], src_ap, gidx_t[:, g0 * 8:(g0 + n) * 8],
                    n * 128, n * 128, ch, queue_num=(g0 // grp) % nqueues)
                for j in range(n):
                    c = g0 + j
                    w, cw = divmod(c, W)
                    sel = selp.tile([128, 128], dt_g, tag="sel")
                    nc.vector.tensor_scalar(
                        sel[:], iota_t[:], loc_t[:, c:c + 1], w_t[:, c:c + 1],
                        op0=eq, op1=mul)
                    if is2[c]:  # dedup chunk: add the second hot
                        selb = selp.tile([128, 128], dt_g, tag="selb")
                        nc.vector.tensor_scalar(
                            selb[:], iota_t[:], locb_t[:, c:c + 1],
                            wb_t[:, c:c + 1], op0=eq, op1=mul)
                        sel2 = selp.tile([128, 128], dt_g, tag="sel2")
                        nc.vector.tensor_tensor(sel2[:], sel[:], selb[:], op=add)
                        sel = sel2
                    if cw == 0:
                        pw = psum.tile([128, ch], f32, tag="win")
                    chunk_cb(pw, sel, gt, j, w, cw, cw == W - 1)
                    if cw == W - 1:
                        win_cb(pw, w)

        def p1_chunk(pw, sel, gt, j, w, cw, last):
            nc.tensor.matmul(pw[:], sel[:], gt[:, j, :],
                             start=(cw == 0), stop=last)

        def gemm_bias(src_t, dst_psum):
            """dst[v/e, co] = src^T blocks @ W.T + 1^T b (K=1 bias matmul)."""
            for k in range(KT):
                nc.tensor.matmul(dst_psum[:], src_t[:, k, :], wt_t[:, k, :],
                                 start=(k == 0), stop=False)
            nc.tensor.matmul(dst_psum[:], ones1_t[:], bt_t[:],
                             start=False, stop=True)

        def transpose_blocks(pw, tag):
            """psum [128, ch] f32 -> sbuf [128, KT, 128] dt_g transposed."""
            t_w = sbp.tile([128, ch], dt_g, tag=tag + "f", name=tag + "f")
            nc.vector.tensor_copy(t_w[:], pw[:])
            tT_w = sbp.tile([128, KT, 128], dt_g, tag=tag + "T", name=tag + "T")
            for k in range(KT):
                pt = psum.tile([128, 128], dt_g, tag="aux", name="pt")
                nc.tensor.transpose(pt[:], t_w[:, k * 128:(k + 1) * 128],
                                    ident_t[:])
                nc.vector.tensor_copy(tT_w[:, k, :], pt[:])
            return tT_w

        def p1_win(pw, w):
            # window w's edge rows are complete: ship its CCIN slice and
            # immediately AllGather that window's table piece, overlapping
            # the collective with the remaining p1 windows.
            xep = sbp.tile([128, ch], dt_p2, tag="xep", name="xep")
            if post:
                # raw table, scaled for fp8 range; GEMM happens after p2 agg
                nc.vector.tensor_scalar(xep[:], pw[:], float(P2_SCALE), None,
                                        op0=mul)
            else:
                xeT_w = transpose_blocks(pw, "xe")
                pg = psum.tile([128, ch], f32, tag="gemm", name="pg")
                gemm_bias(xeT_w, pg)
                nc.vector.tensor_copy(xep[:], pg[:])
            nc.sync.dma_start(CCIN[w * 128:(w + 1) * 128, :], xep[:])
            if AG_SLICED:
                o = w * P.ncores * 128
                if spmd:
                    nc.gpsimd.collective_compute(
                        "AllGather", mybir.AluOpType.bypass,
                        replica_groups=[list(range(P.ncores))],
                        ins=[CCIN[w * 128:(w + 1) * 128, :]],
                        outs=[CCOUT[o:o + P.ncores * 128, :]])
                else:  # single-core stand-in for the window AllGather
                    nc.sync.dma_start(CCOUT[o:o + 128, :],
                                      CCIN[w * 128:(w + 1) * 128, :])
            elif w == P.NW1 - 1:
                if spmd:
                    nc.gpsimd.collective_compute(
                        "AllGather", mybir.AluOpType.bypass,
                        replica_groups=[list(range(P.ncores))],
                        ins=[CCIN[:]], outs=[CCOUT[:]])
                else:
                    nc.sync.dma_start(CCOUT[0:ner1, :], CCIN[:])

        def p2_chunk(pw, sel, gt, j, w, cw, last):
            nc.tensor.matmul(pw[:], sel[:], gt[:, j, :],
                             start=(cw == 0), stop=last)

        def p2_win(pw, w):
            if post:
                awT = transpose_blocks(pw, "aw")
                po = psum.tile([128, ch], f32, tag="gemm", name="po")
                gemm_bias(awT, po)
                pw = po
            # weights pre-normalized on host: just relu + store
            ow = sbp.tile([128, ch], dt_out, tag="ow", name="ow")
            nc.vector.tensor_scalar(ow[:], pw[:], 1.0, 0.0, op0=mul, op1=mx)
            nc.sync.dma_start(OUT[w * 128:(w + 1) * 128, :], ow[:])

        # chunks that contain any dedup pair need the second sel pass; the
        # union over cores keeps the SPMD program identical on every core.
        is2_1 = np.zeros(P.C1, bool)
        is2_2 = np.zeros(P.C2, bool)
        for k in range(P.ncores):
            is2_1 |= (_pack(P.p1[k][3], P.C1) >= 0).any(axis=0)
            is2_2 |= (_pack(P.p2[k][3], P.C2) >= 0).any(axis=0)

        for _rep in range(reps):
            agg_phase(XT[:], g1idx_t, eloc1_t, wsel1_t, eloc1b_t, wsel1b_t,
                      is2_1, P.C1, P.W1, "g1", p1_chunk, p1_win, dt_p1)

            # phase 2: e2v aggregation (sel weights pre-normalized on host)
            agg_phase(CCOUT[:], g2idx_t, vloc2_t, w2raw_t, vloc2b_t, w2rawb_t,
                      is2_2, P.C2, P.W2, "g2", p2_chunk, p2_win, dt_p2)

    nc.compile()
    return nc


# ------------------------------------------------------------------ runner ---
def make_in_maps(P, X, W, b, bf16=GATHER_BF16, p1_dt=P1_DT, fuse=FUSE):
    npdt = ml_dtypes.bfloat16 if bf16 else np.float32
    np_p1 = _npdt(p1_dt)
    s1 = P1_SCALE if p1_dt == "f8" else 1.0
    s2 = P2_SCALE if fuse == "post" else 1.0
    KT = P.ch // 128
    xt = np.ascontiguousarray((X * s1).astype(np_p1))
    wt = np.ascontiguousarray(
        W.T.reshape(KT, 128, P.ch).transpose(1, 0, 2).astype(npdt))
    bt = np.ascontiguousarray(b.astype(npdt).reshape(1, P.ch))
    iota = np.ascontiguousarray(
        np.broadcast_to(np.arange(128, dtype=npdt), (128, 128)))
    ident = np.eye(128, dtype=npdt)

    def tb(flat, C, s=1.0):
        return _pack(flat, C) / np.float32(s)

    in_maps = []
    for k in range(P.ncores):
        g1, l1, w1, l1b, w1b = P.p1[k]
        g2, l2, w2, l2b, w2b = P.p2[k]
        p1tab = np.ascontiguousarray(np.stack(
            [tb(l1, P.C1), tb(w1, P.C1, s1), tb(l1b, P.C1), tb(w1b, P.C1, s1)],
            axis=1))
        p2tab = np.ascontiguousarray(np.stack(
            [tb(l2, P.C2), tb(w2, P.C2, s2), tb(l2b, P.C2), tb(w2b, P.C2, s2)],
            axis=1))
        in_maps.append({
            "xt": xt, "wt": wt, "bt": bt, "iota": iota, "ident": ident,
            "g1idx": _wrap_idx(g1), "p1tab": p1tab,
            "g2idx": _wrap_idx(g2), "p2tab": p2tab,
        })
    return in_maps


def assemble(P, shards):
    out = np.zeros((P.nv, P.ch), np.float32)
    for k in range(P.ncores):
        vm = P.vmap[k]
        m = vm >= 0
        out[vm[m]] = shards[k][m].astype(np.float32)
    return out


_nc_cache = {}


def kernel(X, W, b, e2v_weight, v_idx, e_idx):
    global _last_results
    from concourse.bass_utils import run_bass_kernel_spmd

    P = make_plan(v_idx, e_idx, e2v_weight)
    key = (P.C1, P.C2, P.W1, P.W2, GATHER_BF16, P1_DT, P2_DT, OUT_DT, FUSE)
    if key not in _nc_cache:
        _nc_cache[key] = build_nc(P)
    nc = _nc_cache[key]
    in_maps = make_in_maps(P, X, W, b)
    res = run_bass_kernel_spmd(nc, in_maps, list(range(P.ncores)), trace=TRACE)
    _last_results = res
    shards = [res.results[k]["out"] for k in range(P.ncores)]
    return assemble(P, shards)

